# revision 19
# baseline (speedup 1.0000x reference)
"""Trainium2 Bass kernel: correlation(11x11,dil=2) -> Conv2d(121->8,3x3,p=1) -> BN -> ReLU.

Hardcoded problem: x0, x1 [B=8, C=128, H=192, W=192] f32. Data-parallel over batch
across 8 NeuronCores (1 sample/core); BN training-mode batch stats via a 16-float
DRAM AllReduce. See kernel() at the bottom for the host-side contract.

Per-core dataflow (row-pair pipelined over 96 pairs):
  DVE    products P[c,(row2,dj,j)] = x0[c,i,j]*x1pad[c,i+2di,j+2dj] (fp16, 1 op per
         in-range (pair,di); out-of-range row pairs are skipped entirely)
  PE     channel-sum via selector matmuls: one-hot stationary col (d%32) in col-group
         (d//32) scatters displacement d's sum to PSUM partition d; accumulating
         matmuls -> corr PSUM [121, 2*192] for rows (2p, 2p+1)
  ACT    drains corr rows into bf16 SBUF ring [121, 6*194] (zero side borders);
         skipped displacement rows stay zero (memzero at bottom edge)
  PE     conv row r: <=9 accumulating matmuls, stationary [121,8] bf16 per (ki,kj),
         rhs = ring row r+ki-1 shifted kj -> PSUM [8, 192]; interleaved between
         selector groups to avoid long PE bursts
  ACT    drains conv rows to bf16 staging (+sum via accum_out), Square pass (+sumsq)
  SYNC   staging -> DRAM scratch (bf16) every 8 rows; after all stores, readback
         into a [128, 2304] SBUF layout (p = o*16 + r%16) overlapped with the
         stats AllReduce
  tail   AllReduce [16] -> k=gamma*rsqrt(var+eps), b=beta-k*mean on 8 partitions
         (conv bias cancels in training-mode BN), replicate (k,b) to 128 partitions
         with a one-hot [8,128] matmul, then one ACT relu(k*x+b) over [128, 2304].

Inputs stream in chunks (24 rows each) so products start ~20us into the kernel;
product buffers are triple-buffered so PE conv bursts don't stall the DVE.
"""
import contextlib

import numpy as np

import concourse.bass as bass
import concourse.mybir as mybir
from concourse.ap import AP
from concourse import bass_utils

B = 8
C = 128
H = W = 192
PATCH = 11
DIL = 2
PAD = (PATCH // 2) * DIL          # 10
D = PATCH * PATCH                 # 121
HP = H + 2 * PAD                  # 212
WP = W + 2 * PAD                  # 212
HW = H * W                        # 36864
EPS = 1e-5
NCORES = 8
NPAIRS = H // 2                   # 96
RING = 6
SLOT = W + 2                      # 194
SROWS = 8                         # staging rows per DMA block
NBLK = H // SROWS                 # 24
PRODSZ = 2 * PATCH * W            # 4224 elements per product buffer (2 rows x 11 dj x 192)
PDEPTH = 3                        # product buffers (triple buffer)
XCH = 24                          # input DMA chunk rows (x0 and x1)
NXC = H // XCH                    # 8 chunks each
RG = H // 16                      # 12 row-groups in the [128, RG*W] BN layout
BNW = RG * W                      # 2304

F16 = mybir.dt.float16
BF16 = mybir.dt.bfloat16
F32 = mybir.dt.float32
MULT = mybir.AluOpType.mult
SUB = mybir.AluOpType.subtract
ADD = mybir.AluOpType.add
ACT_COPY = mybir.ActivationFunctionType.Copy
ACT_SQUARE = mybir.ActivationFunctionType.Square
ACT_SQRT = mybir.ActivationFunctionType.Sqrt
ACT_RELU = mybir.ActivationFunctionType.Relu
AX_X = mybir.AxisListType.X

# (pair, di) product groups whose x1 rows are in range; others are exactly zero.
ACTIVE = [(p, di) for p in range(NPAIRS) for di in range(PATCH)
          if 0 <= p + di - 5 <= NPAIRS - 1]
AIDX = {pd: k for k, pd in enumerate(ACTIVE)}

# One group per pair (the last-consumed one) runs on GPSIMD instead of DVE,
# trimming the DVE critical path; PE waits on the right engine's semaphore.
GPS_OFFLOAD = True
_last_di = {}
for _p, _di in ACTIVE:
    _last_di[_p] = _di
ON_GPS = [GPS_OFFLOAD and di == _last_di[p] for (p, di) in ACTIVE]
# cumulative per-engine completion counts at each group index
VCNT = []
GCNT = []
_v = _g = 0
for _k, _gps in enumerate(ON_GPS):
    if _gps:
        _g += 1
    else:
        _v += 1
    VCNT.append(_v)
    GCNT.append(_g)
NV_DVE = _v
SV_STATS = NV_DVE + 1             # after DVE stats reduce
SV_VAR = NV_DVE + 2               # after DVE mean/var
SV_KB = NV_DVE + 3                # after DVE k/b


def _conv_taps(r):
    return [(ki, kj) for ki in range(3) for kj in range(3) if 0 <= r + ki - 1 < H]


def _skips(p):
    """(nlo, nhi): # of skipped di groups at the low/high end for pair p."""
    nlo = max(0, 5 - p)
    nhi = max(0, p - (NPAIRS - 1 - 5))
    return nlo, nhi


def build_nc():
    nc = bass.Bass("TRN2", num_devices=NCORES)

    x0_d = nc.dram_tensor("x0", [C, HW], F32, kind="ExternalInput")
    x1_d = nc.dram_tensor("x1", [C, HW], F32, kind="ExternalInput")
    convw_d = nc.dram_tensor("convw", [D, 72], F32, kind="ExternalInput")
    gamma_d = nc.dram_tensor("gamma", [8], F32, kind="ExternalInput")
    beta_d = nc.dram_tensor("beta", [8], F32, kind="ExternalInput")
    repl_d = nc.dram_tensor("repl", [8, 128], F32, kind="ExternalInput")
    out_d = nc.dram_tensor("out", [8, HW], F32, kind="ExternalOutput")
    scratch_d = nc.dram_tensor("scratch", [8, HW], BF16, kind="Internal")
    cc_in_d = nc.dram_tensor("cc_in", [16], F32, kind="Internal")
    cc_out_d = nc.dram_tensor("cc_out", [16], F32, kind="Internal",
                              addr_space="Shared")

    with contextlib.ExitStack() as outer:
        ee = outer.enter_context
        bn_s = ee(nc.sbuf_tensor([8, 16], F32))
        gamma_s = ee(nc.sbuf_tensor([8, 1], F32))
        beta_s = ee(nc.sbuf_tensor([8, 1], F32))
        repl_s = ee(nc.sbuf_tensor([8, 128], F32))
        bnin_s = ee(nc.sbuf_tensor([128, BNW], BF16))
        sX0 = ee(nc.semaphore())
        sX1 = ee(nc.semaphore())
        sW = ee(nc.semaphore())
        sV = ee(nc.semaphore())
        sMM = ee(nc.semaphore())
        sDrain = ee(nc.semaphore())
        sConvPS = ee(nc.semaphore())
        sConvDr = ee(nc.semaphore())
        sStore = ee(nc.semaphore())
        sStoreB = ee(nc.semaphore())
        sX = ee(nc.semaphore())
        sTail = ee(nc.semaphore())
        sBnIn = ee(nc.semaphore())
        sRepl = ee(nc.semaphore())
        sBnAct = ee(nc.semaphore())
        sBnOut = ee(nc.semaphore())
        sVG = ee(nc.semaphore())
        with contextlib.ExitStack() as inner:
            ei = inner.enter_context
            x1p_s = ei(nc.sbuf_tensor([C, HP * WP], F16))
            x0_s = ei(nc.sbuf_tensor([C, HW], F16))
            prod_s = ei(nc.sbuf_tensor([C, PDEPTH * PRODSZ], F16))
            selz_s = ei(nc.sbuf_tensor([C, 64], F16))
            convw_s = ei(nc.sbuf_tensor([D, 72], BF16))
            ring_s = ei(nc.sbuf_tensor([D, RING * SLOT], BF16))
            stage_s = ei(nc.sbuf_tensor([8, 2 * SROWS * W], BF16))
            ssum_s = ei(nc.sbuf_tensor([8, H], F32))
            ssq_s = ei(nc.sbuf_tensor([8, H], F32))
            junk_s = ei(nc.sbuf_tensor([8, W], F32))
            cps0 = ei(nc.psum_tensor([C, 2 * W], F32))
            cps1 = ei(nc.psum_tensor([C, 2 * W], F32))
            vps0 = ei(nc.psum_tensor([8, W], F32))
            vps1 = ei(nc.psum_tensor([8, W], F32))
            bn_ps = ei(nc.psum_tensor([128, 2], F32))
            block = ei(nc.Block())
            corr_ps = [cps0, cps1]
            conv_ps = [vps0, vps1]

            # ------------- gpsimd: chunked input cast-DMAs + tail collective
            @block.gpsimd
            def _(g):
                x1v = x1_d.ap().rearrange("c (h w) -> c h w", h=H)

                def x1_chunk(k):
                    dst = AP(x1p_s, (PAD + k * XCH) * WP + PAD,
                             [[HP * WP, C], [WP, XCH], [1, W]])
                    g.dma_start(dst, x1v[:, k * XCH:(k + 1) * XCH, :]
                                ).then_inc(sX1, 16)

                def x0_chunk(k):
                    sl = slice(k * XCH * W, (k + 1) * XCH * W)
                    g.dma_start(x0_s[:, sl], x0_d.ap()[:, sl]).then_inc(sX0, 16)

                # first chunks asap, then constants, then the rest interleaved
                x1_chunk(0)
                x0_chunk(0)
                g.dma_start(convw_s[:], convw_d.ap()).then_inc(sW, 16)
                g.dma_start(gamma_s[:],
                            gamma_d.ap().rearrange("(p one) -> p one", one=1)
                            ).then_inc(sW, 16)
                g.dma_start(beta_s[:],
                            beta_d.ap().rearrange("(p one) -> p one", one=1)
                            ).then_inc(sW, 16)
                g.dma_start(repl_s[:], repl_d.ap()).then_inc(sW, 16)
                for k in range(1, NXC):
                    x1_chunk(k)
                    x0_chunk(k)
                # offloaded product groups (one per pair, consumed last by PE)
                hw0 = hw1 = 0
                for k, (p, di) in enumerate(ACTIVE):
                    if not ON_GPS[k]:
                        continue
                    i = 2 * p
                    need0 = 16 * ((2 * p + 1) // XCH + 1)
                    need1 = 16 * ((2 * (p + di - 5) + 1) // XCH + 1)
                    if need0 > hw0:
                        g.wait_ge(sX0, need0)
                        hw0 = need0
                    if need1 > hw1:
                        g.wait_ge(sX1, need1)
                        hw1 = need1
                    if k >= PDEPTH:
                        g.wait_ge(sMM, k - PDEPTH + 1)
                    buf = (k % PDEPTH) * PRODSZ
                    out = AP(prod_s, buf,
                             [[PDEPTH * PRODSZ, C], [PATCH * W, 2], [W, PATCH], [1, W]])
                    in0 = AP(x0_s, i * W, [[HW, C], [W, 2], [0, PATCH], [1, W]])
                    in1 = AP(x1p_s, (i + 2 * di) * WP,
                             [[HP * WP, C], [WP, 2], [DIL, PATCH], [1, W]])
                    g.tensor_tensor(out, in0, in1, MULT).then_inc(sVG, 1)
                # tail: AllReduce of per-core (sum, sumsq)
                g.wait_ge(sTail, 16)
                g.collective_compute(
                    "AllReduce", ADD,
                    replica_groups=[list(range(NCORES))],
                    ins=[cc_in_d.ap()], outs=[cc_out_d.ap()],
                ).then_inc(sTail, 1)

            # ------------- vector: memsets, products, stats/k/b math
            @block.vector
            def _(v):
                # x1p border zeros only (interior fully written by DMA chunks);
                # same-engine program order puts these before any product.
                v.memset(x1p_s[:, 0:PAD * WP], 0.0)
                v.memset(x1p_s[:, (PAD + H) * WP:HP * WP], 0.0)
                v.memset(AP(x1p_s, PAD * WP, [[HP * WP, C], [WP, H], [1, PAD]]),
                         0.0)
                v.memset(AP(x1p_s, PAD * WP + PAD + W,
                            [[HP * WP, C], [WP, H], [1, PAD]]), 0.0)
                v.memset(ring_s[:], 0.0)
                v.memset(selz_s[:, 0:32], 0.0)
                v.memset(selz_s[:, 32:33], 1.0)
                v.memset(selz_s[:, 33:64], 0.0)
                hw0 = hw1 = 0
                for k, (p, di) in enumerate(ACTIVE):
                    if ON_GPS[k]:
                        continue
                    i = 2 * p
                    need0 = 16 * ((2 * p + 1) // XCH + 1)
                    need1 = 16 * ((2 * (p + di - 5) + 1) // XCH + 1)
                    if need0 > hw0:
                        v.wait_ge(sX0, need0)
                        hw0 = need0
                    if need1 > hw1:
                        v.wait_ge(sX1, need1)
                        hw1 = need1
                    if k >= PDEPTH:
                        v.wait_ge(sMM, k - PDEPTH + 1)
                    buf = (k % PDEPTH) * PRODSZ
                    out = AP(prod_s, buf,
                             [[PDEPTH * PRODSZ, C], [PATCH * W, 2], [W, PATCH], [1, W]])
                    in0 = AP(x0_s, i * W, [[HW, C], [W, 2], [0, PATCH], [1, W]])
                    in1 = AP(x1p_s, (i + 2 * di) * WP,
                             [[HP * WP, C], [WP, 2], [DIL, PATCH], [1, W]])
                    v.tensor_tensor(out, in0, in1, MULT).then_inc(sV, 1)
                # stats reduce
                v.wait_ge(sConvDr, 2 * H)
                v.tensor_reduce(bn_s[:, 0:1], ssum_s[:], AX_X, ADD)
                v.tensor_reduce(bn_s[:, 1:2], ssq_s[:], AX_X, ADD).then_inc(sV, 1)
                # mean/var after collective readback (self-sem chains the
                # same-engine RAW dependencies through the DVE pipeline)
                v.wait_ge(sTail, 33)
                ninv = 1.0 / float(B * HW)
                v.tensor_scalar_mul(bn_s[:, 4:5], bn_s[:, 2:3], ninv)
                v.tensor_scalar_mul(bn_s[:, 5:6], bn_s[:, 3:4], ninv).then_inc(sX, 1)
                v.wait_ge(sX, 1)
                v.tensor_tensor(bn_s[:, 6:7], bn_s[:, 4:5], bn_s[:, 4:5],
                                MULT).then_inc(sX, 1)
                v.wait_ge(sX, 2)
                v.tensor_tensor(bn_s[:, 7:8], bn_s[:, 5:6], bn_s[:, 6:7],
                                SUB).then_inc(sX, 1)
                v.wait_ge(sX, 3)
                v.tensor_scalar_add(bn_s[:, 7:8], bn_s[:, 7:8],
                                    EPS).then_inc(sV, 1)
                # k, b after ACT sqrt
                v.wait_ge(sTail, 34)
                v.wait_ge(sW, 64)   # gamma/beta loaded (long since)
                v.reciprocal(bn_s[:, 9:10], bn_s[:, 8:9]).then_inc(sX, 1)
                v.wait_ge(sX, 4)
                v.tensor_tensor(bn_s[:, 10:11], gamma_s[:], bn_s[:, 9:10],
                                MULT).then_inc(sX, 1)
                v.wait_ge(sX, 5)
                v.tensor_tensor(bn_s[:, 13:14], bn_s[:, 10:11], bn_s[:, 4:5],
                                MULT).then_inc(sX, 1)
                v.wait_ge(sX, 6)
                v.tensor_tensor(bn_s[:, 11:12], beta_s[:], bn_s[:, 13:14],
                                SUB).then_inc(sV, 1)

            # ------------- tensor: selector matmuls + conv matmuls + k/b repl
            @block.tensor
            def _(t):
                conv_started = [False]

                def emit_conv(r):
                    if not conv_started[0]:
                        t.wait_ge(sW, 64)   # convw (and other W-group DMAs) done
                        conv_started[0] = True
                    t.wait_ge(sDrain, min(r + 2, H))
                    if r >= 2:
                        t.wait_ge(sConvDr, 2 * (r - 1))
                    pp = conv_ps[r % 2]
                    taps = _conv_taps(r)
                    mm = None
                    for n, (ki, kj) in enumerate(taps):
                        src = r + ki - 1
                        wcol = (ki * 3 + kj) * 8
                        lhsT = convw_s[:, wcol:wcol + 8]
                        rhs = AP(ring_s, (src % RING) * SLOT + kj,
                                 [[RING * SLOT, D], [1, W]])
                        mm = t.matmul(pp[:, :], lhsT, rhs,
                                      start=(n == 0), stop=(n == len(taps) - 1))
                    mm.then_inc(sConvPS, 1)

                for p in range(NPAIRS):
                    if p >= 2:
                        t.wait_ge(sDrain, 2 * p - 2)   # corr psum pp drained
                    started = set()
                    nlo, nhi = _skips(p)
                    dmin, dmax = PATCH * nlo, PATCH * (PATCH - nhi) - 1
                    # last computed displacement per PSUM col-group closes its
                    # accumulation group
                    last_per_cg = {}
                    for d in range(dmin, dmax + 1):
                        last_per_cg[d // 32] = d
                    for di in range(nlo, PATCH - nhi):
                        k = AIDX[(p, di)]
                        if ON_GPS[k]:
                            t.wait_ge(sVG, GCNT[k])
                        else:
                            t.wait_ge(sV, VCNT[k])
                        if di == nlo:
                            # col-groups with no computed displacement still get
                            # drained (full-range PSUM read): write zeros via a
                            # matmul with the all-zero selector as stationary.
                            # After the sV wait, x0_s chunk 0 is loaded, so the
                            # (ignored, x0) rhs values are finite.
                            for cg in range(4):
                                if not (dmin <= 32 * cg + 31 and 32 * cg <= dmax):
                                    t.matmul(corr_ps[p % 2][32 * cg:32 * (cg + 1), :],
                                             selz_s[:, 0:32], x0_s[:, 0:2 * W],
                                             start=True, stop=True,
                                             tile_position=(0, 32 * cg))
                        buf = (k % PDEPTH) * PRODSZ
                        mm = None
                        for dj in range(PATCH):
                            d = di * PATCH + dj
                            cg, m = d // 32, d % 32
                            sel = selz_s[:, 32 - m:64 - m]
                            rhs = AP(prod_s, buf + dj * W,
                                     [[PDEPTH * PRODSZ, C], [PATCH * W, 2], [1, W]])
                            out = corr_ps[p % 2][32 * cg:32 * (cg + 1), :]
                            mm = t.matmul(out, sel, rhs,
                                          start=(cg not in started),
                                          stop=(d == last_per_cg[cg]),
                                          tile_position=(0, 32 * cg))
                            started.add(cg)
                        mm.then_inc(sMM, 1)
                        # interleave conv rows mid-pair to avoid PE bursts
                        if di == nlo + 4:
                            r = 2 * p - 4
                            if 0 <= r:
                                emit_conv(r)
                    r = 2 * p - 3
                    if 0 <= r:
                        emit_conv(r)
                for r in range(2 * NPAIRS - 4, H):
                    emit_conv(r)
                # k,b [8,2] -> [128,2] replication via one-hot [8,128] matmul
                t.wait_ge(sV, SV_KB)
                t.matmul(bn_ps[:, 0:2], repl_s[:], bn_s[:, 10:12],
                         start=True, stop=True).then_inc(sRepl, 1)

            # ------------- scalar: corr drains, conv drains + stats, sqrt
            @block.scalar
            def _(s):
                def conv_drain(r):
                    s.wait_ge(sConvPS, r + 1)
                    if r >= 1:
                        s.wait_ge(sConvDr, 2 * r)   # order junk/stage WAW
                    if r % SROWS == 0 and r >= 2 * SROWS:
                        k = r // SROWS
                        s.wait_ge(sStore if k % 2 == 0 else sStoreB,
                                  16 * (k // 2))
                    base = ((r // SROWS) % 2) * SROWS * W + (r % SROWS) * W
                    s.activation(stage_s[:, base:base + W], conv_ps[r % 2][:, :],
                                 ACT_COPY,
                                 accum_out=ssum_s[:, r:r + 1]).then_inc(sConvDr, 1)
                    s.activation(junk_s[:], conv_ps[r % 2][:, :], ACT_SQUARE,
                                 accum_out=ssq_s[:, r:r + 1]).then_inc(sConvDr, 1)

                mm_done = 0
                for p in range(NPAIRS):
                    i = 2 * p
                    nlo, nhi = _skips(p)
                    mm_done += PATCH - nlo - nhi
                    s.wait_ge(sMM, mm_done)
                    for k in range(2):
                        r = i + k
                        if r >= RING:
                            s.wait_ge(sConvPS, r - 4)   # ring slot reuse
                        dst = AP(ring_s, (r % RING) * SLOT + 1,
                                 [[RING * SLOT, D], [1, W]])
                        s.activation(dst, corr_ps[p % 2][0:D, k * W:(k + 1) * W],
                                     ACT_COPY).then_inc(sDrain, 1)
                    for r in (2 * p - 4, 2 * p - 3):
                        if 0 <= r:
                            conv_drain(r)
                for r in range(2 * NPAIRS - 4, H):
                    conv_drain(r)
                # sqrt(var + eps)
                s.wait_ge(sV, SV_VAR)
                s.activation(bn_s[:, 8:9], bn_s[:, 7:8],
                             ACT_SQRT).then_inc(sTail, 1)

            # ------------- sync: staging stores, stats DMAs, BN readback
            @block.sync
            def _(sy):
                for k in range(NBLK):
                    sy.wait_ge(sConvDr, 2 * SROWS * (k + 1))
                    src = stage_s[:, (k % 2) * SROWS * W:(k % 2 + 1) * SROWS * W]
                    dst = scratch_d.ap()[:, k * SROWS * W:(k + 1) * SROWS * W]
                    sy.dma_start(dst, src).then_inc(sStore if k % 2 == 0 else sStoreB,
                                                    16)
                sy.wait_ge(sV, SV_STATS)
                sy.dma_start(cc_in_d.ap().rearrange("(p two) -> p two", two=2),
                             bn_s[:, 0:2]).then_inc(sTail, 16)
                # readback into [128, 2304] BN layout, overlapped with AllReduce
                scr4 = scratch_d.ap().rearrange("o (rg g j) -> o g rg j",
                                                g=16, j=W)
                for o in range(8):
                    dst = bnin_s[16 * o:16 * (o + 1), :].rearrange(
                        "p (rg j) -> p rg j", j=W)
                    sy.dma_start(dst, scr4[o]).then_inc(sBnIn, 16)
                sy.wait_ge(sTail, 17)
                sy.dma_start(bn_s[:, 2:4],
                             cc_out_d.ap().rearrange("(p two) -> p two", two=2)
                             ).then_inc(sTail, 16)

        # ------------- BN apply tail: reuses freed arena space
        with contextlib.ExitStack() as bstack:
            eb = bstack.enter_context
            bn2_s = eb(nc.sbuf_tensor([128, 2], F32))
            bnout_s = eb(nc.sbuf_tensor([128, BNW], F32))
            block2 = eb(nc.Block())

            @block2.scalar
            def _(s):
                s.wait_ge(sRepl, 1)
                s.activation(bn2_s[:], bn_ps[:, 0:2],
                             ACT_COPY).then_inc(sRepl, 1)
                s.wait_ge(sRepl, 2)
                s.wait_ge(sBnIn, 16 * 8)
                s.activation(bnout_s[:], bnin_s[:], ACT_RELU,
                             bias=bn2_s[:, 1:2],
                             scale=bn2_s[:, 0:1]).then_inc(sBnAct, 1)

            @block2.sync
            def _(sy):
                sy.wait_ge(sBnAct, 1)
                out4 = out_d.ap().rearrange("o (rg g j) -> o g rg j",
                                            g=16, j=W)
                for o in range(8):
                    src = bnout_s[16 * o:16 * (o + 1), :].rearrange(
                        "p (rg j) -> p rg j", j=W)
                    sy.dma_start(out4[o], src).then_inc(sBnOut, 16)

    nc.finalize()
    return nc


_NC_CACHE = None
LAST_EXEC_NS = None
LAST_RES = None


def kernel(x0, x1, conv_w, conv_b, gamma, beta):
    """Full inputs -> full output [8, 8, 192, 192] f32.

    conv_b is intentionally unused: training-mode BatchNorm removes any constant
    per-channel shift (mean' = mean + b exactly cancels it).
    """
    global _NC_CACHE
    x0 = np.ascontiguousarray(np.asarray(x0, dtype=np.float32))
    x1 = np.ascontiguousarray(np.asarray(x1, dtype=np.float32))
    conv_w = np.asarray(conv_w, dtype=np.float32)
    gamma = np.ascontiguousarray(np.asarray(gamma, dtype=np.float32))
    beta = np.ascontiguousarray(np.asarray(beta, dtype=np.float32))

    # lhsT layout [d, (ki, kj, o)]
    convw_l = np.ascontiguousarray(conv_w.transpose(1, 2, 3, 0).reshape(D, 72))
    # one-hot replication matrix: repl[k, m] = 1 iff k == m // 16
    repl = np.zeros((8, 128), dtype=np.float32)
    for kk in range(8):
        repl[kk, 16 * kk:16 * (kk + 1)] = 1.0

    if _NC_CACHE is None:
        _NC_CACHE = build_nc()
    nc = _NC_CACHE

    in_maps = []
    for c in range(NCORES):
        in_maps.append({
            "x0": np.ascontiguousarray(x0[c].reshape(C, HW)),
            "x1": np.ascontiguousarray(x1[c].reshape(C, HW)),
            "convw": convw_l,
            "gamma": gamma,
            "beta": beta,
            "repl": repl,
        })
    import os
    trace = bool(os.environ.get("KERNEL_TRACE"))
    kw = {}
    if trace:
        kw = dict(trace=True, trace_cores=[0])
    res = bass_utils.run_bass_kernel_spmd(nc, in_maps, core_ids=list(range(NCORES)),
                                          **kw)
    global LAST_EXEC_NS, LAST_RES
    LAST_RES = res
    LAST_EXEC_NS = res.exec_time_ns
    out = np.stack([res.results[c]["out"].reshape(8, H, W) for c in range(NCORES)])
    return out.astype(np.float32)


if __name__ == "__main__":
    rng = np.random.default_rng(0)
    x0 = rng.standard_normal((B, C, H, W), dtype=np.float32)
    x1 = rng.standard_normal((B, C, H, W), dtype=np.float32)
    conv_w = (rng.standard_normal((8, D, 3, 3), dtype=np.float32) * 0.05)
    conv_b = (rng.standard_normal((8,), dtype=np.float32) * 0.05)
    gamma = np.ones(8, dtype=np.float32)
    beta = np.zeros(8, dtype=np.float32)
    out = kernel(x0=x0, x1=x1, conv_w=conv_w, conv_b=conv_b, gamma=gamma, beta=beta)
    print("kernel out:", out.shape, out.dtype, float(np.abs(out).max()))


# revision 20
# speedup vs baseline: 1.0955x; 1.0955x over previous
"""Trainium2 Bass kernel: correlation(11x11,dil=2) -> Conv2d(121->8,3x3,p=1) -> BN -> ReLU.

Hardcoded problem: x0, x1 [B=8, C=128, H=192, W=192] f32. Data-parallel over batch
across 8 NeuronCores (1 sample/core); BN training-mode batch stats via a 16-float
DRAM AllReduce. See kernel() at the bottom for the host-side contract.

Per-core dataflow (row-pair pipelined over 96 pairs):
  DVE    products P[c,(row2,dj,j)] = x0[c,i,j]*x1pad[c,i+2di,j+2dj] (fp16, 1 op per
         in-range (pair,di); out-of-range row pairs are skipped entirely)
  PE     channel-sum via selector matmuls: one-hot stationary col (d%32) in col-group
         (d//32) scatters displacement d's sum to PSUM partition d; accumulating
         matmuls -> corr PSUM [121, 2*192] for rows (2p, 2p+1)
  ACT    drains corr rows into bf16 SBUF ring [121, 6*194] (zero side borders);
         skipped displacement rows stay zero (memzero at bottom edge)
  PE     conv row r: <=9 accumulating matmuls, stationary [121,8] bf16 per (ki,kj),
         rhs = ring row r+ki-1 shifted kj -> PSUM [8, 192]; interleaved between
         selector groups to avoid long PE bursts
  ACT    drains conv rows to bf16 staging (+sum via accum_out), Square pass (+sumsq)
  SYNC   staging -> DRAM scratch (bf16) every 8 rows; after all stores, readback
         into a [128, 2304] SBUF layout (p = o*16 + r%16) overlapped with the
         stats AllReduce
  tail   AllReduce [16] -> k=gamma*rsqrt(var+eps), b=beta-k*mean on 8 partitions
         (conv bias cancels in training-mode BN), replicate (k,b) to 128 partitions
         with a one-hot [8,128] matmul, then one ACT relu(k*x+b) over [128, 2304].

Inputs stream in chunks (24 rows each) so products start ~20us into the kernel;
product buffers are triple-buffered so PE conv bursts don't stall the DVE.
"""
import contextlib

import numpy as np

import concourse.bass as bass
import concourse.mybir as mybir
from concourse.ap import AP
from concourse import bass_utils

B = 8
C = 128
H = W = 192
PATCH = 11
DIL = 2
PAD = (PATCH // 2) * DIL          # 10
D = PATCH * PATCH                 # 121
HP = H + 2 * PAD                  # 212
WP = W + 2 * PAD                  # 212
HW = H * W                        # 36864
EPS = 1e-5
NCORES = 8
NPAIRS = H // 2                   # 96
RING = 6
SLOT = W + 2                      # 194
SROWS = 8                         # staging rows per DMA block
NBLK = H // SROWS                 # 24
PRODSZ = 2 * PATCH * W            # 4224 elements per product buffer (2 rows x 11 dj x 192)
PDEPTH = 3                        # product buffers (triple buffer)
XCH = 24                          # input DMA chunk rows (x0 and x1)
NXC = H // XCH                    # 8 chunks each
RG = H // 16                      # 12 row-groups in the [128, RG*W] BN layout
BNW = RG * W                      # 2304

F16 = mybir.dt.float16
BF16 = mybir.dt.bfloat16
F32 = mybir.dt.float32
MULT = mybir.AluOpType.mult
SUB = mybir.AluOpType.subtract
ADD = mybir.AluOpType.add
ACT_COPY = mybir.ActivationFunctionType.Copy
ACT_SQUARE = mybir.ActivationFunctionType.Square
ACT_SQRT = mybir.ActivationFunctionType.Sqrt
ACT_RELU = mybir.ActivationFunctionType.Relu
AX_X = mybir.AxisListType.X

# (pair, di) product groups whose x1 rows are in range; others are exactly zero.
ACTIVE = [(p, di) for p in range(NPAIRS) for di in range(PATCH)
          if 0 <= p + di - 5 <= NPAIRS - 1]
AIDX = {pd: k for k, pd in enumerate(ACTIVE)}
NV_ACT = len(ACTIVE)              # 1026
SV_STATS = NV_ACT + 1             # after DVE stats reduce
SV_VAR = NV_ACT + 2               # after DVE mean/var
SV_KB = NV_ACT + 3                # after DVE k/b


def _conv_taps(r):
    return [(ki, kj) for ki in range(3) for kj in range(3) if 0 <= r + ki - 1 < H]


def _skips(p):
    """(nlo, nhi): # of skipped di groups at the low/high end for pair p."""
    nlo = max(0, 5 - p)
    nhi = max(0, p - (NPAIRS - 1 - 5))
    return nlo, nhi


def build_nc():
    nc = bass.Bass("TRN2", num_devices=NCORES)

    x0_d = nc.dram_tensor("x0", [C, HW], F32, kind="ExternalInput")
    x1_d = nc.dram_tensor("x1", [C, HW], F32, kind="ExternalInput")
    convw_d = nc.dram_tensor("convw", [D, 72], F32, kind="ExternalInput")
    gamma_d = nc.dram_tensor("gamma", [8], F32, kind="ExternalInput")
    beta_d = nc.dram_tensor("beta", [8], F32, kind="ExternalInput")
    repl_d = nc.dram_tensor("repl", [8, 128], F32, kind="ExternalInput")
    out_d = nc.dram_tensor("out", [8, HW], F32, kind="ExternalOutput")
    scratch_d = nc.dram_tensor("scratch", [8, HW], BF16, kind="Internal")
    cc_in_d = nc.dram_tensor("cc_in", [16], F32, kind="Internal")
    cc_out_d = nc.dram_tensor("cc_out", [16], F32, kind="Internal",
                              addr_space="Shared")

    with contextlib.ExitStack() as outer:
        ee = outer.enter_context
        bn_s = ee(nc.sbuf_tensor([8, 16], F32))
        gamma_s = ee(nc.sbuf_tensor([8, 1], F32))
        beta_s = ee(nc.sbuf_tensor([8, 1], F32))
        repl_s = ee(nc.sbuf_tensor([8, 128], F32))
        bnin_s = ee(nc.sbuf_tensor([128, BNW], BF16))
        sX0 = ee(nc.semaphore())
        sX1 = ee(nc.semaphore())
        sW = ee(nc.semaphore())
        sV = ee(nc.semaphore())
        sMM = ee(nc.semaphore())
        sDrain = ee(nc.semaphore())
        sConvPS = ee(nc.semaphore())
        sConvDr = ee(nc.semaphore())
        sStore = ee(nc.semaphore())
        sStoreB = ee(nc.semaphore())
        sX = ee(nc.semaphore())
        sTail = ee(nc.semaphore())
        sBnIn = ee(nc.semaphore())
        sRepl = ee(nc.semaphore())
        sBnAct = ee(nc.semaphore())
        sBnOut = ee(nc.semaphore())
        with contextlib.ExitStack() as inner:
            ei = inner.enter_context
            x1p_s = ei(nc.sbuf_tensor([C, HP * WP], F16))
            x0_s = ei(nc.sbuf_tensor([C, HW], F16))
            prod_s = ei(nc.sbuf_tensor([C, PDEPTH * PRODSZ], F16))
            selz_s = ei(nc.sbuf_tensor([C, 64], F16))
            convw_s = ei(nc.sbuf_tensor([D, 72], BF16))
            ring_s = ei(nc.sbuf_tensor([D, RING * SLOT], BF16))
            stage_s = ei(nc.sbuf_tensor([8, 2 * SROWS * W], BF16))
            ssum_s = ei(nc.sbuf_tensor([8, H], F32))
            ssq_s = ei(nc.sbuf_tensor([8, H], F32))
            junk_s = ei(nc.sbuf_tensor([8, W], F32))
            cps0 = ei(nc.psum_tensor([C, 2 * W], F32))
            cps1 = ei(nc.psum_tensor([C, 2 * W], F32))
            vps0 = ei(nc.psum_tensor([8, W], F32))
            vps1 = ei(nc.psum_tensor([8, W], F32))
            bn_ps = ei(nc.psum_tensor([128, 2], F32))
            block = ei(nc.Block())
            corr_ps = [cps0, cps1]
            conv_ps = [vps0, vps1]

            # ------------- gpsimd: chunked input cast-DMAs + tail collective
            @block.gpsimd
            def _(g):
                x1v = x1_d.ap().rearrange("c (h w) -> c h w", h=H)

                def x1_chunk(k):
                    dst = AP(x1p_s, (PAD + k * XCH) * WP + PAD,
                             [[HP * WP, C], [WP, XCH], [1, W]])
                    g.dma_start(dst, x1v[:, k * XCH:(k + 1) * XCH, :]
                                ).then_inc(sX1, 16)

                def x0_chunk(k):
                    sl = slice(k * XCH * W, (k + 1) * XCH * W)
                    g.dma_start(x0_s[:, sl], x0_d.ap()[:, sl]).then_inc(sX0, 16)

                # first chunks asap, then constants, then the rest interleaved
                x1_chunk(0)
                x0_chunk(0)
                g.dma_start(convw_s[:], convw_d.ap()).then_inc(sW, 16)
                g.dma_start(gamma_s[:],
                            gamma_d.ap().rearrange("(p one) -> p one", one=1)
                            ).then_inc(sW, 16)
                g.dma_start(beta_s[:],
                            beta_d.ap().rearrange("(p one) -> p one", one=1)
                            ).then_inc(sW, 16)
                g.dma_start(repl_s[:], repl_d.ap()).then_inc(sW, 16)
                for k in range(1, NXC):
                    x1_chunk(k)
                    x0_chunk(k)
                # tail: AllReduce of per-core (sum, sumsq)
                g.wait_ge(sTail, 16)
                g.collective_compute(
                    "AllReduce", ADD,
                    replica_groups=[list(range(NCORES))],
                    ins=[cc_in_d.ap()], outs=[cc_out_d.ap()],
                ).then_inc(sTail, 1)

            # ------------- vector: memsets, products, stats/k/b math
            @block.vector
            def _(v):
                # x1p border zeros only (interior fully written by DMA chunks);
                # same-engine program order puts these before any product.
                v.memset(x1p_s[:, 0:PAD * WP], 0.0)
                v.memset(x1p_s[:, (PAD + H) * WP:HP * WP], 0.0)
                v.memset(AP(x1p_s, PAD * WP, [[HP * WP, C], [WP, H], [1, PAD]]),
                         0.0)
                v.memset(AP(x1p_s, PAD * WP + PAD + W,
                            [[HP * WP, C], [WP, H], [1, PAD]]), 0.0)
                v.memset(ring_s[:], 0.0)
                v.memset(selz_s[:, 0:32], 0.0)
                v.memset(selz_s[:, 32:33], 1.0)
                v.memset(selz_s[:, 33:64], 0.0)
                hw0 = hw1 = 0
                for k, (p, di) in enumerate(ACTIVE):
                    i = 2 * p
                    need0 = 16 * ((2 * p + 1) // XCH + 1)
                    need1 = 16 * ((2 * (p + di - 5) + 1) // XCH + 1)
                    if need0 > hw0:
                        v.wait_ge(sX0, need0)
                        hw0 = need0
                    if need1 > hw1:
                        v.wait_ge(sX1, need1)
                        hw1 = need1
                    if k >= PDEPTH:
                        v.wait_ge(sMM, k - PDEPTH + 1)
                    buf = (k % PDEPTH) * PRODSZ
                    out = AP(prod_s, buf,
                             [[PDEPTH * PRODSZ, C], [PATCH * W, 2], [W, PATCH], [1, W]])
                    in0 = AP(x0_s, i * W, [[HW, C], [W, 2], [0, PATCH], [1, W]])
                    in1 = AP(x1p_s, (i + 2 * di) * WP,
                             [[HP * WP, C], [WP, 2], [DIL, PATCH], [1, W]])
                    v.tensor_tensor(out, in0, in1, MULT).then_inc(sV, 1)
                # stats reduce
                v.wait_ge(sConvDr, 2 * H)
                v.tensor_reduce(bn_s[:, 0:1], ssum_s[:], AX_X, ADD)
                v.tensor_reduce(bn_s[:, 1:2], ssq_s[:], AX_X, ADD).then_inc(sV, 1)
                # mean/var after collective readback (self-sem chains the
                # same-engine RAW dependencies through the DVE pipeline)
                v.wait_ge(sTail, 33)
                ninv = 1.0 / float(B * HW)
                v.tensor_scalar_mul(bn_s[:, 4:5], bn_s[:, 2:3], ninv)
                v.tensor_scalar_mul(bn_s[:, 5:6], bn_s[:, 3:4], ninv).then_inc(sX, 1)
                v.wait_ge(sX, 1)
                v.tensor_tensor(bn_s[:, 6:7], bn_s[:, 4:5], bn_s[:, 4:5],
                                MULT).then_inc(sX, 1)
                v.wait_ge(sX, 2)
                v.tensor_tensor(bn_s[:, 7:8], bn_s[:, 5:6], bn_s[:, 6:7],
                                SUB).then_inc(sX, 1)
                v.wait_ge(sX, 3)
                v.tensor_scalar_add(bn_s[:, 7:8], bn_s[:, 7:8],
                                    EPS).then_inc(sV, 1)
                # k, b after ACT sqrt
                v.wait_ge(sTail, 34)
                v.wait_ge(sW, 64)   # gamma/beta loaded (long since)
                v.reciprocal(bn_s[:, 9:10], bn_s[:, 8:9]).then_inc(sX, 1)
                v.wait_ge(sX, 4)
                v.tensor_tensor(bn_s[:, 10:11], gamma_s[:], bn_s[:, 9:10],
                                MULT).then_inc(sX, 1)
                v.wait_ge(sX, 5)
                v.tensor_tensor(bn_s[:, 13:14], bn_s[:, 10:11], bn_s[:, 4:5],
                                MULT).then_inc(sX, 1)
                v.wait_ge(sX, 6)
                v.tensor_tensor(bn_s[:, 11:12], beta_s[:], bn_s[:, 13:14],
                                SUB).then_inc(sV, 1)

            # ------------- tensor: selector matmuls + conv matmuls + k/b repl
            @block.tensor
            def _(t):
                conv_started = [False]

                def emit_conv(r):
                    if not conv_started[0]:
                        t.wait_ge(sW, 64)   # convw (and other W-group DMAs) done
                        conv_started[0] = True
                    t.wait_ge(sDrain, min(r + 2, H))
                    if r >= 2:
                        t.wait_ge(sConvDr, 2 * (r - 1))
                    pp = conv_ps[r % 2]
                    taps = _conv_taps(r)
                    mm = None
                    for n, (ki, kj) in enumerate(taps):
                        src = r + ki - 1
                        wcol = (ki * 3 + kj) * 8
                        lhsT = convw_s[:, wcol:wcol + 8]
                        rhs = AP(ring_s, (src % RING) * SLOT + kj,
                                 [[RING * SLOT, D], [1, W]])
                        mm = t.matmul(pp[:, :], lhsT, rhs,
                                      start=(n == 0), stop=(n == len(taps) - 1))
                    mm.then_inc(sConvPS, 1)

                for p in range(NPAIRS):
                    if p >= 2:
                        t.wait_ge(sDrain, 2 * p - 2)   # corr psum pp drained
                    started = set()
                    nlo, nhi = _skips(p)
                    dmin, dmax = PATCH * nlo, PATCH * (PATCH - nhi) - 1
                    # last computed displacement per PSUM col-group closes its
                    # accumulation group
                    last_per_cg = {}
                    for d in range(dmin, dmax + 1):
                        last_per_cg[d // 32] = d
                    for di in range(nlo, PATCH - nhi):
                        k = AIDX[(p, di)]
                        t.wait_ge(sV, k + 1)
                        if di == nlo:
                            # col-groups with no computed displacement still get
                            # drained (full-range PSUM read): write zeros via a
                            # matmul with the all-zero selector as stationary.
                            # After the sV wait, x0_s chunk 0 is loaded, so the
                            # (ignored, x0) rhs values are finite.
                            for cg in range(4):
                                if not (dmin <= 32 * cg + 31 and 32 * cg <= dmax):
                                    t.matmul(corr_ps[p % 2][32 * cg:32 * (cg + 1), :],
                                             selz_s[:, 0:32], x0_s[:, 0:2 * W],
                                             start=True, stop=True,
                                             tile_position=(0, 32 * cg))
                        buf = (k % PDEPTH) * PRODSZ
                        mm = None
                        for dj in range(PATCH):
                            d = di * PATCH + dj
                            cg, m = d // 32, d % 32
                            sel = selz_s[:, 32 - m:64 - m]
                            rhs = AP(prod_s, buf + dj * W,
                                     [[PDEPTH * PRODSZ, C], [PATCH * W, 2], [1, W]])
                            out = corr_ps[p % 2][32 * cg:32 * (cg + 1), :]
                            mm = t.matmul(out, sel, rhs,
                                          start=(cg not in started),
                                          stop=(d == last_per_cg[cg]),
                                          tile_position=(0, 32 * cg))
                            started.add(cg)
                        mm.then_inc(sMM, 1)
                        # interleave conv rows mid-pair to avoid PE bursts
                        if di == nlo + 4:
                            r = 2 * p - 4
                            if 0 <= r:
                                emit_conv(r)
                    r = 2 * p - 3
                    if 0 <= r:
                        emit_conv(r)
                for r in range(2 * NPAIRS - 4, H):
                    emit_conv(r)
                # k,b [8,2] -> [128,2] replication via one-hot [8,128] matmul
                t.wait_ge(sV, SV_KB)
                t.matmul(bn_ps[:, 0:2], repl_s[:], bn_s[:, 10:12],
                         start=True, stop=True).then_inc(sRepl, 1)

            # ------------- scalar: corr drains, conv drains + stats, sqrt
            @block.scalar
            def _(s):
                def conv_drain(r):
                    s.wait_ge(sConvPS, r + 1)
                    if r >= 1:
                        s.wait_ge(sConvDr, 2 * r)   # order junk/stage WAW
                    if r % SROWS == 0 and r >= 2 * SROWS:
                        k = r // SROWS
                        s.wait_ge(sStore if k % 2 == 0 else sStoreB,
                                  16 * (k // 2))
                    base = ((r // SROWS) % 2) * SROWS * W + (r % SROWS) * W
                    s.activation(stage_s[:, base:base + W], conv_ps[r % 2][:, :],
                                 ACT_COPY,
                                 accum_out=ssum_s[:, r:r + 1]).then_inc(sConvDr, 1)
                    s.activation(junk_s[:], conv_ps[r % 2][:, :], ACT_SQUARE,
                                 accum_out=ssq_s[:, r:r + 1]).then_inc(sConvDr, 1)

                mm_done = 0
                for p in range(NPAIRS):
                    i = 2 * p
                    nlo, nhi = _skips(p)
                    mm_done += PATCH - nlo - nhi
                    s.wait_ge(sMM, mm_done)
                    for k in range(2):
                        r = i + k
                        if r >= RING:
                            s.wait_ge(sConvPS, r - 4)   # ring slot reuse
                        dst = AP(ring_s, (r % RING) * SLOT + 1,
                                 [[RING * SLOT, D], [1, W]])
                        s.activation(dst, corr_ps[p % 2][0:D, k * W:(k + 1) * W],
                                     ACT_COPY).then_inc(sDrain, 1)
                    for r in (2 * p - 4, 2 * p - 3):
                        if 0 <= r:
                            conv_drain(r)
                for r in range(2 * NPAIRS - 4, H):
                    conv_drain(r)
                # sqrt(var + eps)
                s.wait_ge(sV, SV_VAR)
                s.activation(bn_s[:, 8:9], bn_s[:, 7:8],
                             ACT_SQRT).then_inc(sTail, 1)

            # ------------- sync: staging stores, stats DMAs, BN readback
            @block.sync
            def _(sy):
                for k in range(NBLK):
                    sy.wait_ge(sConvDr, 2 * SROWS * (k + 1))
                    src = stage_s[:, (k % 2) * SROWS * W:(k % 2 + 1) * SROWS * W]
                    dst = scratch_d.ap()[:, k * SROWS * W:(k + 1) * SROWS * W]
                    sy.dma_start(dst, src).then_inc(sStore if k % 2 == 0 else sStoreB,
                                                    16)
                sy.wait_ge(sV, SV_STATS)
                sy.dma_start(cc_in_d.ap().rearrange("(p two) -> p two", two=2),
                             bn_s[:, 0:2]).then_inc(sTail, 16)
                # readback into [128, 2304] BN layout, overlapped with AllReduce
                scr4 = scratch_d.ap().rearrange("o (rg g j) -> o g rg j",
                                                g=16, j=W)
                for o in range(8):
                    dst = bnin_s[16 * o:16 * (o + 1), :].rearrange(
                        "p (rg j) -> p rg j", j=W)
                    sy.dma_start(dst, scr4[o]).then_inc(sBnIn, 16)
                sy.wait_ge(sTail, 17)
                sy.dma_start(bn_s[:, 2:4],
                             cc_out_d.ap().rearrange("(p two) -> p two", two=2)
                             ).then_inc(sTail, 16)

        # ------------- BN apply tail: reuses freed arena space
        with contextlib.ExitStack() as bstack:
            eb = bstack.enter_context
            bn2_s = eb(nc.sbuf_tensor([128, 2], F32))
            bnout_s = eb(nc.sbuf_tensor([128, BNW], F32))
            block2 = eb(nc.Block())

            @block2.scalar
            def _(s):
                s.wait_ge(sRepl, 1)
                s.activation(bn2_s[:], bn_ps[:, 0:2],
                             ACT_COPY).then_inc(sRepl, 1)
                s.wait_ge(sRepl, 2)
                s.wait_ge(sBnIn, 16 * 8)
                s.activation(bnout_s[:], bnin_s[:], ACT_RELU,
                             bias=bn2_s[:, 1:2],
                             scale=bn2_s[:, 0:1]).then_inc(sBnAct, 1)

            @block2.sync
            def _(sy):
                sy.wait_ge(sBnAct, 1)
                out4 = out_d.ap().rearrange("o (rg g j) -> o g rg j",
                                            g=16, j=W)
                for o in range(8):
                    src = bnout_s[16 * o:16 * (o + 1), :].rearrange(
                        "p (rg j) -> p rg j", j=W)
                    sy.dma_start(out4[o], src).then_inc(sBnOut, 16)

    nc.finalize()
    return nc


_NC_CACHE = None
LAST_EXEC_NS = None
LAST_RES = None


def kernel(x0, x1, conv_w, conv_b, gamma, beta):
    """Full inputs -> full output [8, 8, 192, 192] f32.

    conv_b is intentionally unused: training-mode BatchNorm removes any constant
    per-channel shift (mean' = mean + b exactly cancels it).
    """
    global _NC_CACHE
    x0 = np.ascontiguousarray(np.asarray(x0, dtype=np.float32))
    x1 = np.ascontiguousarray(np.asarray(x1, dtype=np.float32))
    conv_w = np.asarray(conv_w, dtype=np.float32)
    gamma = np.ascontiguousarray(np.asarray(gamma, dtype=np.float32))
    beta = np.ascontiguousarray(np.asarray(beta, dtype=np.float32))

    # lhsT layout [d, (ki, kj, o)]
    convw_l = np.ascontiguousarray(conv_w.transpose(1, 2, 3, 0).reshape(D, 72))
    # one-hot replication matrix: repl[k, m] = 1 iff k == m // 16
    repl = np.zeros((8, 128), dtype=np.float32)
    for kk in range(8):
        repl[kk, 16 * kk:16 * (kk + 1)] = 1.0

    if _NC_CACHE is None:
        _NC_CACHE = build_nc()
    nc = _NC_CACHE

    in_maps = []
    for c in range(NCORES):
        in_maps.append({
            "x0": np.ascontiguousarray(x0[c].reshape(C, HW)),
            "x1": np.ascontiguousarray(x1[c].reshape(C, HW)),
            "convw": convw_l,
            "gamma": gamma,
            "beta": beta,
            "repl": repl,
        })
    import os
    trace = bool(os.environ.get("KERNEL_TRACE"))
    kw = {}
    if trace:
        kw = dict(trace=True, trace_cores=[0])
    res = bass_utils.run_bass_kernel_spmd(nc, in_maps, core_ids=list(range(NCORES)),
                                          **kw)
    global LAST_EXEC_NS, LAST_RES
    LAST_RES = res
    LAST_EXEC_NS = res.exec_time_ns
    out = np.stack([res.results[c]["out"].reshape(8, H, W) for c in range(NCORES)])
    return out.astype(np.float32)


if __name__ == "__main__":
    rng = np.random.default_rng(0)
    x0 = rng.standard_normal((B, C, H, W), dtype=np.float32)
    x1 = rng.standard_normal((B, C, H, W), dtype=np.float32)
    conv_w = (rng.standard_normal((8, D, 3, 3), dtype=np.float32) * 0.05)
    conv_b = (rng.standard_normal((8,), dtype=np.float32) * 0.05)
    gamma = np.ones(8, dtype=np.float32)
    beta = np.zeros(8, dtype=np.float32)
    out = kernel(x0=x0, x1=x1, conv_w=conv_w, conv_b=conv_b, gamma=gamma, beta=beta)
    print("kernel out:", out.shape, out.dtype, float(np.abs(out).max()))


# revision 26
# speedup vs baseline: 1.3139x; 1.1993x over previous
"""Trainium2 Bass kernel: correlation(11x11,dil=2) -> Conv2d(121->8,3x3,p=1) -> BN -> ReLU.

Hardcoded problem: x0, x1 [B=8, C=128, H=192, W=192] f32. Data-parallel over batch
across 8 NeuronCores (1 sample/core); BN training-mode batch stats via a 16-float
DRAM AllReduce. See kernel() at the bottom for the host-side contract.

Per-core dataflow (row-pair pipelined over 96 pairs):
  DVE    products P[c,(row2,dj,j)] = x0[c,i,j]*x1pad[c,i+2di,j+2dj] (fp16, 1 op per
         in-range (pair,di); out-of-range row pairs are skipped entirely)
  PE     channel-sum via selector matmuls: one-hot stationary col (d%32) in col-group
         (d//32) scatters displacement d's sum to PSUM partition d; accumulating
         matmuls -> corr PSUM [121, 2*192] for rows (2p, 2p+1)
  ACT    drains corr rows into bf16 SBUF ring [121, 6*194] (zero side borders);
         skipped displacement rows stay zero (memzero at bottom edge)
  PE     conv row r: <=9 accumulating matmuls, stationary [121,8] bf16 per (ki,kj),
         rhs = ring row r+ki-1 shifted kj -> PSUM [8, 192]; interleaved between
         selector groups to avoid long PE bursts
  ACT    drains conv rows to bf16 staging (+sum via accum_out), Square pass (+sumsq)
  SYNC   staging -> DRAM scratch (bf16) every 8 rows; after all stores, readback
         into a [128, 2304] SBUF layout (p = o*16 + r%16) overlapped with the
         stats AllReduce
  tail   AllReduce [16] -> k=gamma*rsqrt(var+eps), b=beta-k*mean on 8 partitions
         (conv bias cancels in training-mode BN), replicate (k,b) to 128 partitions
         with a one-hot [8,128] matmul, then one ACT relu(k*x+b) over [128, 2304].

Inputs stream in chunks (24 rows each) so products start ~20us into the kernel;
product buffers are triple-buffered so PE conv bursts don't stall the DVE.
"""
import contextlib

import numpy as np

import concourse.bass as bass
import concourse.mybir as mybir
from concourse.ap import AP
from concourse import bass_utils

B = 8
C = 128
H = W = 192
PATCH = 11
DIL = 2
PAD = (PATCH // 2) * DIL          # 10
D = PATCH * PATCH                 # 121
HP = H + 2 * PAD                  # 212
WP = W + 2 * PAD                  # 212
HW = H * W                        # 36864
EPS = 1e-5
NCORES = 8
NPAIRS = H // 2                   # 96
RING = 6
SLOT = W + 2                      # 194
SROWS = 8                         # staging rows per DMA block
NBLK = H // SROWS                 # 24
PRODSZ = 2 * PATCH * W            # 4224 elements per product buffer (2 rows x 11 dj x 192)
PDEPTH = 3                        # product buffers (triple buffer)
# input DMA chunk row boundaries (smaller first chunks -> earlier first product)
XBOUNDS = [0, 12, 24, 48, 72, 96, 120, 144, 168, 192]
NXC = len(XBOUNDS) - 1


def _chunk_of(row):
    for ci in range(NXC):
        if row < XBOUNDS[ci + 1]:
            return ci
    raise ValueError(row)
RG = H // 16                      # 12 row-groups in the [128, RG*W] BN layout
BNW = RG * W                      # 2304

F16 = mybir.dt.float16
BF16 = mybir.dt.bfloat16
F32 = mybir.dt.float32
MULT = mybir.AluOpType.mult
SUB = mybir.AluOpType.subtract
ADD = mybir.AluOpType.add
ACT_COPY = mybir.ActivationFunctionType.Copy
ACT_SQUARE = mybir.ActivationFunctionType.Square
ACT_SQRT = mybir.ActivationFunctionType.Sqrt
ACT_RELU = mybir.ActivationFunctionType.Relu
AX_X = mybir.AxisListType.X

# (pair, di) product groups whose x1 rows are in range; others are exactly zero.
ACTIVE = [(p, di) for p in range(NPAIRS) for di in range(PATCH)
          if 0 <= p + di - 5 <= NPAIRS - 1]
AIDX = {pd: k for k, pd in enumerate(ACTIVE)}
NV_ACT = len(ACTIVE)              # 1026
SV_STATS = NV_ACT + 1             # after DVE stats reduce
SV_VAR = NV_ACT + 2               # after DVE mean/var
SV_KB = NV_ACT + 3                # after DVE k/b


def _conv_taps(r):
    return [(ki, kj) for ki in range(3) for kj in range(3) if 0 <= r + ki - 1 < H]


def _skips(p):
    """(nlo, nhi): # of skipped di groups at the low/high end for pair p."""
    nlo = max(0, 5 - p)
    nhi = max(0, p - (NPAIRS - 1 - 5))
    return nlo, nhi


def build_nc():
    nc = bass.Bass("TRN2", num_devices=NCORES)

    # fp16 inputs (host-cast): halves input HBM traffic vs f32 + DMA cast
    x0_d = nc.dram_tensor("x0", [C, HW], F16, kind="ExternalInput")
    x1_d = nc.dram_tensor("x1", [C, HW], F16, kind="ExternalInput")
    convw_d = nc.dram_tensor("convw", [D, 72], F32, kind="ExternalInput")
    gamma_d = nc.dram_tensor("gamma", [8], F32, kind="ExternalInput")
    beta_d = nc.dram_tensor("beta", [8], F32, kind="ExternalInput")
    repl_d = nc.dram_tensor("repl", [8, 128], F32, kind="ExternalInput")
    out_d = nc.dram_tensor("out", [8, HW], F32, kind="ExternalOutput")
    scratch_d = nc.dram_tensor("scratch", [8, HW], BF16, kind="Internal")
    cc_in_d = nc.dram_tensor("cc_in", [16], F32, kind="Internal")
    cc_out_d = nc.dram_tensor("cc_out", [16], F32, kind="Internal",
                              addr_space="Shared")

    with contextlib.ExitStack() as outer:
        ee = outer.enter_context
        bn_s = ee(nc.sbuf_tensor([8, 16], F32))
        gamma_s = ee(nc.sbuf_tensor([8, 1], F32))
        beta_s = ee(nc.sbuf_tensor([8, 1], F32))
        repl_s = ee(nc.sbuf_tensor([8, 128], F32))
        bnin_s = ee(nc.sbuf_tensor([128, BNW], BF16))
        sX0 = ee(nc.semaphore())
        sX1 = ee(nc.semaphore())
        sW = ee(nc.semaphore())
        sV = ee(nc.semaphore())
        sMM = ee(nc.semaphore())
        sDrain = ee(nc.semaphore())
        sConvPS = ee(nc.semaphore())
        sConvDr = ee(nc.semaphore())
        sStore = ee(nc.semaphore())
        sStoreB = ee(nc.semaphore())
        sX = ee(nc.semaphore())
        sTail = ee(nc.semaphore())
        sBnIn = ee(nc.semaphore())
        sRepl = ee(nc.semaphore())
        sBnAct = ee(nc.semaphore())
        sBnOut = ee(nc.semaphore())
        with contextlib.ExitStack() as inner:
            ei = inner.enter_context
            x1p_s = ei(nc.sbuf_tensor([C, HP * WP], F16))
            x0_s = ei(nc.sbuf_tensor([C, HW], F16))
            prod_s = ei(nc.sbuf_tensor([C, PDEPTH * PRODSZ], F16))
            selz_s = ei(nc.sbuf_tensor([C, 64], F16))
            convw_s = ei(nc.sbuf_tensor([D, 72], BF16))
            ring_s = ei(nc.sbuf_tensor([D, RING * SLOT], BF16))
            stage_s = ei(nc.sbuf_tensor([8, 2 * SROWS * W], BF16))
            ssum_s = ei(nc.sbuf_tensor([8, H], F32))
            ssq_s = ei(nc.sbuf_tensor([8, H], F32))
            junk_s = ei(nc.sbuf_tensor([8, W], F32))
            cps0 = ei(nc.psum_tensor([C, 2 * W], F32))
            cps1 = ei(nc.psum_tensor([C, 2 * W], F32))
            vps0 = ei(nc.psum_tensor([8, W], F32))
            vps1 = ei(nc.psum_tensor([8, W], F32))
            bn_ps = ei(nc.psum_tensor([128, 2], F32))
            block = ei(nc.Block())
            corr_ps = [cps0, cps1]
            conv_ps = [vps0, vps1]

            # ------------- gpsimd: chunked input cast-DMAs + tail collective
            @block.gpsimd
            def _(g):
                x1v = x1_d.ap().rearrange("c (h w) -> c h w", h=H)

                def x1_chunk(k):
                    r0, r1 = XBOUNDS[k], XBOUNDS[k + 1]
                    dst = AP(x1p_s, (PAD + r0) * WP + PAD,
                             [[HP * WP, C], [WP, r1 - r0], [1, W]])
                    g.dma_start(dst, x1v[:, r0:r1, :]).then_inc(sX1, 16)

                def x0_chunk(k):
                    sl = slice(XBOUNDS[k] * W, XBOUNDS[k + 1] * W)
                    g.dma_start(x0_s[:, sl], x0_d.ap()[:, sl]).then_inc(sX0, 16)

                # first chunks asap, then constants, then the rest interleaved
                x1_chunk(0)
                x0_chunk(0)
                g.dma_start(convw_s[:], convw_d.ap()).then_inc(sW, 16)
                g.dma_start(gamma_s[:],
                            gamma_d.ap().rearrange("(p one) -> p one", one=1)
                            ).then_inc(sW, 16)
                g.dma_start(beta_s[:],
                            beta_d.ap().rearrange("(p one) -> p one", one=1)
                            ).then_inc(sW, 16)
                g.dma_start(repl_s[:], repl_d.ap()).then_inc(sW, 16)
                for k in range(1, NXC):
                    x1_chunk(k)
                    x0_chunk(k)
                # tail: AllReduce of per-core (sum, sumsq)
                g.wait_ge(sTail, 16)
                g.collective_compute(
                    "AllReduce", ADD,
                    replica_groups=[list(range(NCORES))],
                    ins=[cc_in_d.ap()], outs=[cc_out_d.ap()],
                ).then_inc(sTail, 1)

            # ------------- vector: memsets, products, stats/k/b math
            @block.vector
            def _(v):
                # x1p border zeros only (interior fully written by DMA chunks);
                # same-engine program order puts these before any product.
                v.memset(x1p_s[:, 0:PAD * WP], 0.0)
                v.memset(x1p_s[:, (PAD + H) * WP:HP * WP], 0.0)
                v.memset(AP(x1p_s, PAD * WP, [[HP * WP, C], [WP, H], [1, PAD]]),
                         0.0)
                v.memset(AP(x1p_s, PAD * WP + PAD + W,
                            [[HP * WP, C], [WP, H], [1, PAD]]), 0.0)
                v.memset(ring_s[:], 0.0)
                v.memset(selz_s[:, 0:32], 0.0)
                v.memset(selz_s[:, 32:33], 1.0)
                v.memset(selz_s[:, 33:64], 0.0)
                hw0 = hw1 = 0
                for k, (p, di) in enumerate(ACTIVE):
                    i = 2 * p
                    need0 = 16 * (_chunk_of(2 * p + 1) + 1)
                    need1 = 16 * (_chunk_of(2 * (p + di - 5) + 1) + 1)
                    if need0 > hw0:
                        v.wait_ge(sX0, need0)
                        hw0 = need0
                    if need1 > hw1:
                        v.wait_ge(sX1, need1)
                        hw1 = need1
                    if k >= PDEPTH:
                        v.wait_ge(sMM, k - PDEPTH + 1)
                    buf = (k % PDEPTH) * PRODSZ
                    out = AP(prod_s, buf,
                             [[PDEPTH * PRODSZ, C], [PATCH * W, 2], [W, PATCH], [1, W]])
                    in0 = AP(x0_s, i * W, [[HW, C], [W, 2], [0, PATCH], [1, W]])
                    in1 = AP(x1p_s, (i + 2 * di) * WP,
                             [[HP * WP, C], [WP, 2], [DIL, PATCH], [1, W]])
                    v.tensor_tensor(out, in0, in1, MULT).then_inc(sV, 1)
                # stats reduce
                v.wait_ge(sConvDr, 2 * H)
                v.tensor_reduce(bn_s[:, 0:1], ssum_s[:], AX_X, ADD)
                v.tensor_reduce(bn_s[:, 1:2], ssq_s[:], AX_X, ADD).then_inc(sV, 1)
                # mean/var after collective readback (self-sem chains the
                # same-engine RAW dependencies through the DVE pipeline)
                v.wait_ge(sTail, 33)
                ninv = 1.0 / float(B * HW)
                v.tensor_scalar_mul(bn_s[:, 4:5], bn_s[:, 2:3], ninv)
                v.tensor_scalar_mul(bn_s[:, 5:6], bn_s[:, 3:4], ninv).then_inc(sX, 1)
                v.wait_ge(sX, 1)
                v.tensor_tensor(bn_s[:, 6:7], bn_s[:, 4:5], bn_s[:, 4:5],
                                MULT).then_inc(sX, 1)
                v.wait_ge(sX, 2)
                v.tensor_tensor(bn_s[:, 7:8], bn_s[:, 5:6], bn_s[:, 6:7],
                                SUB).then_inc(sX, 1)
                v.wait_ge(sX, 3)
                v.tensor_scalar_add(bn_s[:, 7:8], bn_s[:, 7:8],
                                    EPS).then_inc(sV, 1)
                # k, b after ACT sqrt
                v.wait_ge(sTail, 34)
                v.wait_ge(sW, 64)   # gamma/beta loaded (long since)
                v.reciprocal(bn_s[:, 9:10], bn_s[:, 8:9]).then_inc(sX, 1)
                v.wait_ge(sX, 4)
                v.tensor_tensor(bn_s[:, 10:11], gamma_s[:], bn_s[:, 9:10],
                                MULT).then_inc(sX, 1)
                v.wait_ge(sX, 5)
                v.tensor_tensor(bn_s[:, 13:14], bn_s[:, 10:11], bn_s[:, 4:5],
                                MULT).then_inc(sX, 1)
                v.wait_ge(sX, 6)
                v.tensor_tensor(bn_s[:, 11:12], beta_s[:], bn_s[:, 13:14],
                                SUB).then_inc(sV, 1)

            # ------------- tensor: selector matmuls + conv matmuls + k/b repl
            @block.tensor
            def _(t):
                conv_started = [False]

                def emit_conv(r):
                    if not conv_started[0]:
                        t.wait_ge(sW, 64)   # convw (and other W-group DMAs) done
                        conv_started[0] = True
                    t.wait_ge(sDrain, min(r + 2, H))
                    if r >= 2:
                        t.wait_ge(sConvDr, 2 * (r - 1))
                    pp = conv_ps[r % 2]
                    taps = _conv_taps(r)
                    mm = None
                    for n, (ki, kj) in enumerate(taps):
                        src = r + ki - 1
                        wcol = (ki * 3 + kj) * 8
                        lhsT = convw_s[:, wcol:wcol + 8]
                        rhs = AP(ring_s, (src % RING) * SLOT + kj,
                                 [[RING * SLOT, D], [1, W]])
                        mm = t.matmul(pp[:, :], lhsT, rhs,
                                      start=(n == 0), stop=(n == len(taps) - 1))
                    mm.then_inc(sConvPS, 1)

                for p in range(NPAIRS):
                    if p >= 2:
                        t.wait_ge(sDrain, 2 * p - 2)   # corr psum pp drained
                    started = set()
                    nlo, nhi = _skips(p)
                    dmin, dmax = PATCH * nlo, PATCH * (PATCH - nhi) - 1
                    # last computed displacement per PSUM col-group closes its
                    # accumulation group
                    last_per_cg = {}
                    for d in range(dmin, dmax + 1):
                        last_per_cg[d // 32] = d
                    for di in range(nlo, PATCH - nhi):
                        k = AIDX[(p, di)]
                        t.wait_ge(sV, k + 1)
                        if di == nlo:
                            # col-groups with no computed displacement still get
                            # drained (full-range PSUM read): write zeros via a
                            # matmul with the all-zero selector as stationary.
                            # After the sV wait, x0_s chunk 0 is loaded, so the
                            # (ignored, x0) rhs values are finite.
                            for cg in range(4):
                                if not (dmin <= 32 * cg + 31 and 32 * cg <= dmax):
                                    t.matmul(corr_ps[p % 2][32 * cg:32 * (cg + 1), :],
                                             selz_s[:, 0:32], x0_s[:, 0:2 * W],
                                             start=True, stop=True,
                                             tile_position=(0, 32 * cg))
                        buf = (k % PDEPTH) * PRODSZ
                        mm = None
                        for dj in range(PATCH):
                            d = di * PATCH + dj
                            cg, m = d // 32, d % 32
                            sel = selz_s[:, 32 - m:64 - m]
                            rhs = AP(prod_s, buf + dj * W,
                                     [[PDEPTH * PRODSZ, C], [PATCH * W, 2], [1, W]])
                            out = corr_ps[p % 2][32 * cg:32 * (cg + 1), :]
                            mm = t.matmul(out, sel, rhs,
                                          start=(cg not in started),
                                          stop=(d == last_per_cg[cg]),
                                          tile_position=(0, 32 * cg))
                            started.add(cg)
                        mm.then_inc(sMM, 1)
                        # interleave conv rows mid-pair to avoid PE bursts
                        if di == nlo + 4:
                            r = 2 * p - 4
                            if 0 <= r:
                                emit_conv(r)
                    r = 2 * p - 3
                    if 0 <= r:
                        emit_conv(r)
                for r in range(2 * NPAIRS - 4, H):
                    emit_conv(r)
                # k,b [8,2] -> [128,2] replication via one-hot [8,128] matmul
                t.wait_ge(sV, SV_KB)
                t.matmul(bn_ps[:, 0:2], repl_s[:], bn_s[:, 10:12],
                         start=True, stop=True).then_inc(sRepl, 1)

            # ------------- scalar: corr drains, conv drains + stats, sqrt
            @block.scalar
            def _(s):
                def conv_drain(r):
                    s.wait_ge(sConvPS, r + 1)
                    if r >= 1:
                        s.wait_ge(sConvDr, 2 * r)   # order junk/stage WAW
                    if r % SROWS == 0 and r >= 2 * SROWS:
                        k = r // SROWS
                        s.wait_ge(sStore if k % 2 == 0 else sStoreB,
                                  16 * (k // 2))
                    base = ((r // SROWS) % 2) * SROWS * W + (r % SROWS) * W
                    s.activation(stage_s[:, base:base + W], conv_ps[r % 2][:, :],
                                 ACT_COPY,
                                 accum_out=ssum_s[:, r:r + 1]).then_inc(sConvDr, 1)
                    s.activation(junk_s[:], conv_ps[r % 2][:, :], ACT_SQUARE,
                                 accum_out=ssq_s[:, r:r + 1]).then_inc(sConvDr, 1)

                mm_done = 0
                for p in range(NPAIRS):
                    i = 2 * p
                    nlo, nhi = _skips(p)
                    mm_done += PATCH - nlo - nhi
                    s.wait_ge(sMM, mm_done)
                    for k in range(2):
                        r = i + k
                        if r >= RING:
                            s.wait_ge(sConvPS, r - 4)   # ring slot reuse
                        dst = AP(ring_s, (r % RING) * SLOT + 1,
                                 [[RING * SLOT, D], [1, W]])
                        s.activation(dst, corr_ps[p % 2][0:D, k * W:(k + 1) * W],
                                     ACT_COPY).then_inc(sDrain, 1)
                    for r in (2 * p - 4, 2 * p - 3):
                        if 0 <= r:
                            conv_drain(r)
                for r in range(2 * NPAIRS - 4, H):
                    conv_drain(r)
                # sqrt(var + eps)
                s.wait_ge(sV, SV_VAR)
                s.activation(bn_s[:, 8:9], bn_s[:, 7:8],
                             ACT_SQRT).then_inc(sTail, 1)

            # ------------- sync: staging stores, stats DMAs, BN readback
            @block.sync
            def _(sy):
                for k in range(NBLK):
                    sy.wait_ge(sConvDr, 2 * SROWS * (k + 1))
                    src = stage_s[:, (k % 2) * SROWS * W:(k % 2 + 1) * SROWS * W]
                    dst = scratch_d.ap()[:, k * SROWS * W:(k + 1) * SROWS * W]
                    sy.dma_start(dst, src).then_inc(sStore if k % 2 == 0 else sStoreB,
                                                    16)
                sy.wait_ge(sV, SV_STATS)
                sy.dma_start(cc_in_d.ap().rearrange("(p two) -> p two", two=2),
                             bn_s[:, 0:2]).then_inc(sTail, 16)
                # readback into [128, 2304] BN layout, overlapped with AllReduce
                scr4 = scratch_d.ap().rearrange("o (rg g j) -> o g rg j",
                                                g=16, j=W)
                for o in range(8):
                    dst = bnin_s[16 * o:16 * (o + 1), :].rearrange(
                        "p (rg j) -> p rg j", j=W)
                    sy.dma_start(dst, scr4[o]).then_inc(sBnIn, 16)
                sy.wait_ge(sTail, 17)
                sy.dma_start(bn_s[:, 2:4],
                             cc_out_d.ap().rearrange("(p two) -> p two", two=2)
                             ).then_inc(sTail, 16)

        # ------------- BN apply tail: reuses freed arena space
        with contextlib.ExitStack() as bstack:
            eb = bstack.enter_context
            bn2_s = eb(nc.sbuf_tensor([128, 2], F32))
            bnout_s = eb(nc.sbuf_tensor([128, BNW], F32))
            block2 = eb(nc.Block())

            out4 = out_d.ap().rearrange("o (rg g j) -> o g rg j", g=16, j=W)

            def out_src(o):
                return bnout_s[16 * o:16 * (o + 1), :].rearrange(
                    "p (rg j) -> p rg j", j=W)

            @block2.scalar
            def _(s):
                s.wait_ge(sRepl, 1)
                s.activation(bn2_s[:], bn_ps[:, 0:2],
                             ACT_COPY).then_inc(sRepl, 1)
                s.wait_ge(sRepl, 2)
                s.wait_ge(sBnIn, 16 * 8)
                s.activation(bnout_s[:], bnin_s[:], ACT_RELU,
                             bias=bn2_s[:, 1:2],
                             scale=bn2_s[:, 0:1]).then_inc(sBnAct, 1)
                # half the output stores issue from the ACT HWDGE ring so the
                # two physical DGE rings drain the result in parallel
                for o in range(4, 8):
                    s.dma_start(out4[o], out_src(o)).then_inc(sBnOut, 16)

            @block2.sync
            def _(sy):
                sy.wait_ge(sBnAct, 1)
                for o in range(4):
                    sy.dma_start(out4[o], out_src(o)).then_inc(sBnOut, 16)

    nc.finalize()
    return nc


_NC_CACHE = None
LAST_EXEC_NS = None
LAST_RES = None


def kernel(x0, x1, conv_w, conv_b, gamma, beta):
    """Full inputs -> full output [8, 8, 192, 192] f32.

    conv_b is intentionally unused: training-mode BatchNorm removes any constant
    per-channel shift (mean' = mean + b exactly cancels it).
    """
    global _NC_CACHE
    x0 = np.asarray(x0, dtype=np.float32).astype(np.float16)
    x1 = np.asarray(x1, dtype=np.float32).astype(np.float16)
    conv_w = np.asarray(conv_w, dtype=np.float32)
    gamma = np.ascontiguousarray(np.asarray(gamma, dtype=np.float32))
    beta = np.ascontiguousarray(np.asarray(beta, dtype=np.float32))

    # lhsT layout [d, (ki, kj, o)]
    convw_l = np.ascontiguousarray(conv_w.transpose(1, 2, 3, 0).reshape(D, 72))
    # one-hot replication matrix: repl[k, m] = 1 iff k == m // 16
    repl = np.zeros((8, 128), dtype=np.float32)
    for kk in range(8):
        repl[kk, 16 * kk:16 * (kk + 1)] = 1.0

    if _NC_CACHE is None:
        _NC_CACHE = build_nc()
    nc = _NC_CACHE

    in_maps = []
    for c in range(NCORES):
        in_maps.append({
            "x0": np.ascontiguousarray(x0[c].reshape(C, HW)),
            "x1": np.ascontiguousarray(x1[c].reshape(C, HW)),
            "convw": convw_l,
            "gamma": gamma,
            "beta": beta,
            "repl": repl,
        })
    import os
    trace = bool(os.environ.get("KERNEL_TRACE"))
    kw = {}
    if trace:
        kw = dict(trace=True, trace_cores=[0])
    res = bass_utils.run_bass_kernel_spmd(nc, in_maps, core_ids=list(range(NCORES)),
                                          **kw)
    global LAST_EXEC_NS, LAST_RES
    LAST_RES = res
    LAST_EXEC_NS = res.exec_time_ns
    out = np.stack([res.results[c]["out"].reshape(8, H, W) for c in range(NCORES)])
    return out.astype(np.float32)


if __name__ == "__main__":
    rng = np.random.default_rng(0)
    x0 = rng.standard_normal((B, C, H, W), dtype=np.float32)
    x1 = rng.standard_normal((B, C, H, W), dtype=np.float32)
    conv_w = (rng.standard_normal((8, D, 3, 3), dtype=np.float32) * 0.05)
    conv_b = (rng.standard_normal((8,), dtype=np.float32) * 0.05)
    gamma = np.ones(8, dtype=np.float32)
    beta = np.zeros(8, dtype=np.float32)
    out = kernel(x0=x0, x1=x1, conv_w=conv_w, conv_b=conv_b, gamma=gamma, beta=beta)
    print("kernel out:", out.shape, out.dtype, float(np.abs(out).max()))


# revision 28
# speedup vs baseline: 1.3159x; 1.0015x over previous
"""Trainium2 Bass kernel: correlation(11x11,dil=2) -> Conv2d(121->8,3x3,p=1) -> BN -> ReLU.

Hardcoded problem: x0, x1 [B=8, C=128, H=192, W=192] f32. Data-parallel over batch
across 8 NeuronCores (1 sample/core); BN training-mode batch stats via a 16-float
DRAM AllReduce. See kernel() at the bottom for the host-side contract.

Per-core dataflow (row-pair pipelined over 96 pairs):
  DVE    products P[c,(row2,dj,j)] = x0[c,i,j]*x1pad[c,i+2di,j+2dj] (fp16, 1 op per
         in-range (pair,di); out-of-range row pairs are skipped entirely)
  PE     channel-sum via selector matmuls: one-hot stationary col (d%32) in col-group
         (d//32) scatters displacement d's sum to PSUM partition d; accumulating
         matmuls -> corr PSUM [121, 2*192] for rows (2p, 2p+1)
  ACT    drains corr rows into bf16 SBUF ring [121, 6*194] (zero side borders);
         skipped displacement rows stay zero (memzero at bottom edge)
  PE     conv row r: <=9 accumulating matmuls, stationary [121,8] bf16 per (ki,kj),
         rhs = ring row r+ki-1 shifted kj -> PSUM [8, 192]; interleaved between
         selector groups to avoid long PE bursts
  ACT    drains conv rows to bf16 staging (+sum via accum_out), Square pass (+sumsq)
  SYNC   staging -> DRAM scratch (bf16) every 8 rows; after all stores, readback
         into a [128, 2304] SBUF layout (p = o*16 + r%16) overlapped with the
         stats AllReduce
  tail   AllReduce [16] -> k=gamma*rsqrt(var+eps), b=beta-k*mean on 8 partitions
         (conv bias cancels in training-mode BN), replicate (k,b) to 128 partitions
         with a one-hot [8,128] matmul, then one ACT relu(k*x+b) over [128, 2304].

Inputs stream in chunks (24 rows each) so products start ~20us into the kernel;
product buffers are triple-buffered so PE conv bursts don't stall the DVE.
"""
import contextlib

import numpy as np

import concourse.bass as bass
import concourse.mybir as mybir
from concourse.ap import AP
from concourse import bass_utils

B = 8
C = 128
H = W = 192
PATCH = 11
DIL = 2
PAD = (PATCH // 2) * DIL          # 10
D = PATCH * PATCH                 # 121
HP = H + 2 * PAD                  # 212
WP = W + 2 * PAD                  # 212
HW = H * W                        # 36864
EPS = 1e-5
NCORES = 8
NPAIRS = H // 2                   # 96
RING = 6
SLOT = W + 2                      # 194
SROWS = 8                         # staging rows per DMA block
NBLK = H // SROWS                 # 24
PRODSZ = 2 * PATCH * W            # 4224 elements per product buffer (2 rows x 11 dj x 192)
PDEPTH = 3                        # product buffers (triple buffer)
# input DMA chunk row boundaries (smaller first chunks -> earlier first product)
XBOUNDS = [0, 12, 24, 48, 72, 96, 120, 144, 168, 192]
NXC = len(XBOUNDS) - 1


def _chunk_of(row):
    for ci in range(NXC):
        if row < XBOUNDS[ci + 1]:
            return ci
    raise ValueError(row)
RG = H // 16                      # 12 row-groups in the [128, RG*W] BN layout
BNW = RG * W                      # 2304

F16 = mybir.dt.float16
BF16 = mybir.dt.bfloat16
F32 = mybir.dt.float32
MULT = mybir.AluOpType.mult
SUB = mybir.AluOpType.subtract
ADD = mybir.AluOpType.add
ACT_COPY = mybir.ActivationFunctionType.Copy
ACT_SQUARE = mybir.ActivationFunctionType.Square
ACT_SQRT = mybir.ActivationFunctionType.Sqrt
ACT_RELU = mybir.ActivationFunctionType.Relu
AX_X = mybir.AxisListType.X

# (pair, di) product groups whose x1 rows are in range; others are exactly zero.
ACTIVE = [(p, di) for p in range(NPAIRS) for di in range(PATCH)
          if 0 <= p + di - 5 <= NPAIRS - 1]
AIDX = {pd: k for k, pd in enumerate(ACTIVE)}
NV_ACT = len(ACTIVE)              # 1026
SV_STATS = NV_ACT + 1             # after DVE stats reduce
SV_VAR = NV_ACT + 2               # after DVE mean/var
SV_KB = NV_ACT + 3                # after DVE k/b


def _conv_taps(r):
    return [(ki, kj) for ki in range(3) for kj in range(3) if 0 <= r + ki - 1 < H]


def _skips(p):
    """(nlo, nhi): # of skipped di groups at the low/high end for pair p."""
    nlo = max(0, 5 - p)
    nhi = max(0, p - (NPAIRS - 1 - 5))
    return nlo, nhi


def build_nc():
    nc = bass.Bass("TRN2", num_devices=NCORES)

    # fp16 inputs (host-cast): halves input HBM traffic vs f32 + DMA cast
    x0_d = nc.dram_tensor("x0", [C, HW], F16, kind="ExternalInput")
    x1_d = nc.dram_tensor("x1", [C, HW], F16, kind="ExternalInput")
    convw_d = nc.dram_tensor("convw", [D, 72], F32, kind="ExternalInput")
    gamma_d = nc.dram_tensor("gamma", [8], F32, kind="ExternalInput")
    beta_d = nc.dram_tensor("beta", [8], F32, kind="ExternalInput")
    repl_d = nc.dram_tensor("repl", [8, 128], F32, kind="ExternalInput")
    out_d = nc.dram_tensor("out", [8, HW], F32, kind="ExternalOutput")
    scratch_d = nc.dram_tensor("scratch", [8, HW], BF16, kind="Internal")
    cc_in_d = nc.dram_tensor("cc_in", [16], F32, kind="Internal")
    cc_out_d = nc.dram_tensor("cc_out", [16], F32, kind="Internal",
                              addr_space="Shared")

    with contextlib.ExitStack() as outer:
        ee = outer.enter_context
        bn_s = ee(nc.sbuf_tensor([8, 16], F32))
        gamma_s = ee(nc.sbuf_tensor([8, 1], F32))
        beta_s = ee(nc.sbuf_tensor([8, 1], F32))
        repl_s = ee(nc.sbuf_tensor([8, 128], F32))
        bnin_s = ee(nc.sbuf_tensor([128, BNW], BF16))
        sX0 = ee(nc.semaphore())
        sX1 = ee(nc.semaphore())
        sW = ee(nc.semaphore())
        sV = ee(nc.semaphore())
        sMM = ee(nc.semaphore())
        sDrain = ee(nc.semaphore())
        sConvPS = ee(nc.semaphore())
        sConvDr = ee(nc.semaphore())
        sStore = ee(nc.semaphore())
        sStoreB = ee(nc.semaphore())
        sX = ee(nc.semaphore())
        sTail = ee(nc.semaphore())
        sBnIn = ee(nc.semaphore())
        sRepl = ee(nc.semaphore())
        sBnAct = ee(nc.semaphore())
        sBnOut = ee(nc.semaphore())
        with contextlib.ExitStack() as inner:
            ei = inner.enter_context
            x1p_s = ei(nc.sbuf_tensor([C, HP * WP], F16))
            x0_s = ei(nc.sbuf_tensor([C, HW], F16))
            prod_s = ei(nc.sbuf_tensor([C, PDEPTH * PRODSZ], F16))
            selz_s = ei(nc.sbuf_tensor([C, 64], F16))
            convw_s = ei(nc.sbuf_tensor([D, 72], BF16))
            ring_s = ei(nc.sbuf_tensor([D, RING * SLOT], BF16))
            stage_s = ei(nc.sbuf_tensor([8, 2 * SROWS * W], BF16))
            ssum_s = ei(nc.sbuf_tensor([8, H], F32))
            ssq_s = ei(nc.sbuf_tensor([8, H], F32))
            junk_s = ei(nc.sbuf_tensor([8, W], F32))
            cps0 = ei(nc.psum_tensor([C, 2 * W], F32))
            cps1 = ei(nc.psum_tensor([C, 2 * W], F32))
            vps0 = ei(nc.psum_tensor([8, W], F32))
            vps1 = ei(nc.psum_tensor([8, W], F32))
            bn_ps = ei(nc.psum_tensor([128, 2], F32))
            block = ei(nc.Block())
            corr_ps = [cps0, cps1]
            conv_ps = [vps0, vps1]

            # ------------- gpsimd: convw cast-DMA + tail collective
            @block.gpsimd
            def _(g):
                # convw needs an f32->bf16 cast, which only SWDGE can do
                g.dma_start(convw_s[:], convw_d.ap()).then_inc(sW, 16)
                # tail: AllReduce of per-core (sum, sumsq)
                g.wait_ge(sTail, 16)
                g.collective_compute(
                    "AllReduce", ADD,
                    replica_groups=[list(range(NCORES))],
                    ins=[cc_in_d.ap()], outs=[cc_out_d.ap()],
                ).then_inc(sTail, 1)

            # ------------- vector: memsets, products, stats/k/b math
            @block.vector
            def _(v):
                # x1p border zeros only (interior fully written by DMA chunks);
                # same-engine program order puts these before any product.
                v.memset(x1p_s[:, 0:PAD * WP], 0.0)
                v.memset(x1p_s[:, (PAD + H) * WP:HP * WP], 0.0)
                v.memset(AP(x1p_s, PAD * WP, [[HP * WP, C], [WP, H], [1, PAD]]),
                         0.0)
                v.memset(AP(x1p_s, PAD * WP + PAD + W,
                            [[HP * WP, C], [WP, H], [1, PAD]]), 0.0)
                v.memset(ring_s[:], 0.0)
                v.memset(selz_s[:, 0:32], 0.0)
                v.memset(selz_s[:, 32:33], 1.0)
                v.memset(selz_s[:, 33:64], 0.0)
                hw0 = hw1 = 0
                for k, (p, di) in enumerate(ACTIVE):
                    i = 2 * p
                    need0 = 16 * (_chunk_of(2 * p + 1) + 1)
                    need1 = 16 * (_chunk_of(2 * (p + di - 5) + 1) + 1)
                    if need0 > hw0:
                        v.wait_ge(sX0, need0)
                        hw0 = need0
                    if need1 > hw1:
                        v.wait_ge(sX1, need1)
                        hw1 = need1
                    if k >= PDEPTH:
                        v.wait_ge(sMM, k - PDEPTH + 1)
                    buf = (k % PDEPTH) * PRODSZ
                    out = AP(prod_s, buf,
                             [[PDEPTH * PRODSZ, C], [PATCH * W, 2], [W, PATCH], [1, W]])
                    in0 = AP(x0_s, i * W, [[HW, C], [W, 2], [0, PATCH], [1, W]])
                    in1 = AP(x1p_s, (i + 2 * di) * WP,
                             [[HP * WP, C], [WP, 2], [DIL, PATCH], [1, W]])
                    v.tensor_tensor(out, in0, in1, MULT).then_inc(sV, 1)
                # stats reduce
                v.wait_ge(sConvDr, 2 * H)
                v.tensor_reduce(bn_s[:, 0:1], ssum_s[:], AX_X, ADD)
                v.tensor_reduce(bn_s[:, 1:2], ssq_s[:], AX_X, ADD).then_inc(sV, 1)
                # mean/var after collective readback (self-sem chains the
                # same-engine RAW dependencies through the DVE pipeline)
                v.wait_ge(sTail, 33)
                ninv = 1.0 / float(B * HW)
                v.tensor_scalar_mul(bn_s[:, 4:5], bn_s[:, 2:3], ninv)
                v.tensor_scalar_mul(bn_s[:, 5:6], bn_s[:, 3:4], ninv).then_inc(sX, 1)
                v.wait_ge(sX, 1)
                v.tensor_tensor(bn_s[:, 6:7], bn_s[:, 4:5], bn_s[:, 4:5],
                                MULT).then_inc(sX, 1)
                v.wait_ge(sX, 2)
                v.tensor_tensor(bn_s[:, 7:8], bn_s[:, 5:6], bn_s[:, 6:7],
                                SUB).then_inc(sX, 1)
                v.wait_ge(sX, 3)
                v.tensor_scalar_add(bn_s[:, 7:8], bn_s[:, 7:8],
                                    EPS).then_inc(sV, 1)
                # k, b after ACT sqrt
                v.wait_ge(sTail, 34)
                v.wait_ge(sW, 64)   # gamma/beta loaded (long since)
                v.reciprocal(bn_s[:, 9:10], bn_s[:, 8:9]).then_inc(sX, 1)
                v.wait_ge(sX, 4)
                v.tensor_tensor(bn_s[:, 10:11], gamma_s[:], bn_s[:, 9:10],
                                MULT).then_inc(sX, 1)
                v.wait_ge(sX, 5)
                v.tensor_tensor(bn_s[:, 13:14], bn_s[:, 10:11], bn_s[:, 4:5],
                                MULT).then_inc(sX, 1)
                v.wait_ge(sX, 6)
                v.tensor_tensor(bn_s[:, 11:12], beta_s[:], bn_s[:, 13:14],
                                SUB).then_inc(sV, 1)

            # ------------- tensor: selector matmuls + conv matmuls + k/b repl
            @block.tensor
            def _(t):
                conv_started = [False]

                def emit_conv(r):
                    if not conv_started[0]:
                        t.wait_ge(sW, 64)   # convw (and other W-group DMAs) done
                        conv_started[0] = True
                    t.wait_ge(sDrain, min(r + 2, H))
                    if r >= 2:
                        t.wait_ge(sConvDr, 2 * (r - 1))
                    pp = conv_ps[r % 2]
                    taps = _conv_taps(r)
                    mm = None
                    for n, (ki, kj) in enumerate(taps):
                        src = r + ki - 1
                        wcol = (ki * 3 + kj) * 8
                        lhsT = convw_s[:, wcol:wcol + 8]
                        rhs = AP(ring_s, (src % RING) * SLOT + kj,
                                 [[RING * SLOT, D], [1, W]])
                        mm = t.matmul(pp[:, :], lhsT, rhs,
                                      start=(n == 0), stop=(n == len(taps) - 1))
                    mm.then_inc(sConvPS, 1)

                for p in range(NPAIRS):
                    if p >= 2:
                        t.wait_ge(sDrain, 2 * p - 2)   # corr psum pp drained
                    started = set()
                    nlo, nhi = _skips(p)
                    dmin, dmax = PATCH * nlo, PATCH * (PATCH - nhi) - 1
                    # last computed displacement per PSUM col-group closes its
                    # accumulation group
                    last_per_cg = {}
                    for d in range(dmin, dmax + 1):
                        last_per_cg[d // 32] = d
                    for di in range(nlo, PATCH - nhi):
                        k = AIDX[(p, di)]
                        t.wait_ge(sV, k + 1)
                        if di == nlo:
                            # col-groups with no computed displacement still get
                            # drained (full-range PSUM read): write zeros via a
                            # matmul with the all-zero selector as stationary.
                            # After the sV wait, x0_s chunk 0 is loaded, so the
                            # (ignored, x0) rhs values are finite.
                            for cg in range(4):
                                if not (dmin <= 32 * cg + 31 and 32 * cg <= dmax):
                                    t.matmul(corr_ps[p % 2][32 * cg:32 * (cg + 1), :],
                                             selz_s[:, 0:32], x0_s[:, 0:2 * W],
                                             start=True, stop=True,
                                             tile_position=(0, 32 * cg))
                        buf = (k % PDEPTH) * PRODSZ
                        mm = None
                        for dj in range(PATCH):
                            d = di * PATCH + dj
                            cg, m = d // 32, d % 32
                            sel = selz_s[:, 32 - m:64 - m]
                            rhs = AP(prod_s, buf + dj * W,
                                     [[PDEPTH * PRODSZ, C], [PATCH * W, 2], [1, W]])
                            out = corr_ps[p % 2][32 * cg:32 * (cg + 1), :]
                            mm = t.matmul(out, sel, rhs,
                                          start=(cg not in started),
                                          stop=(d == last_per_cg[cg]),
                                          tile_position=(0, 32 * cg))
                            started.add(cg)
                        mm.then_inc(sMM, 1)
                        # interleave conv rows mid-pair to avoid PE bursts
                        if di == nlo + 4:
                            r = 2 * p - 4
                            if 0 <= r:
                                emit_conv(r)
                    r = 2 * p - 3
                    if 0 <= r:
                        emit_conv(r)
                for r in range(2 * NPAIRS - 4, H):
                    emit_conv(r)
                # k,b [8,2] -> [128,2] replication via one-hot [8,128] matmul
                t.wait_ge(sV, SV_KB)
                t.matmul(bn_ps[:, 0:2], repl_s[:], bn_s[:, 10:12],
                         start=True, stop=True).then_inc(sRepl, 1)

            # ------------- scalar: corr drains, conv drains + stats, sqrt
            @block.scalar
            def _(s):
                def conv_drain(r):
                    s.wait_ge(sConvPS, r + 1)
                    if r >= 1:
                        s.wait_ge(sConvDr, 2 * r)   # order junk/stage WAW
                    if r % SROWS == 0 and r >= 2 * SROWS:
                        k = r // SROWS
                        s.wait_ge(sStore if k % 2 == 0 else sStoreB,
                                  16 * (k // 2))
                    base = ((r // SROWS) % 2) * SROWS * W + (r % SROWS) * W
                    s.activation(stage_s[:, base:base + W], conv_ps[r % 2][:, :],
                                 ACT_COPY,
                                 accum_out=ssum_s[:, r:r + 1]).then_inc(sConvDr, 1)
                    s.activation(junk_s[:], conv_ps[r % 2][:, :], ACT_SQUARE,
                                 accum_out=ssq_s[:, r:r + 1]).then_inc(sConvDr, 1)

                mm_done = 0
                for p in range(NPAIRS):
                    i = 2 * p
                    nlo, nhi = _skips(p)
                    mm_done += PATCH - nlo - nhi
                    s.wait_ge(sMM, mm_done)
                    for k in range(2):
                        r = i + k
                        if r >= RING:
                            s.wait_ge(sConvPS, r - 4)   # ring slot reuse
                        dst = AP(ring_s, (r % RING) * SLOT + 1,
                                 [[RING * SLOT, D], [1, W]])
                        s.activation(dst, corr_ps[p % 2][0:D, k * W:(k + 1) * W],
                                     ACT_COPY).then_inc(sDrain, 1)
                    for r in (2 * p - 4, 2 * p - 3):
                        if 0 <= r:
                            conv_drain(r)
                for r in range(2 * NPAIRS - 4, H):
                    conv_drain(r)
                # sqrt(var + eps)
                s.wait_ge(sV, SV_VAR)
                s.activation(bn_s[:, 8:9], bn_s[:, 7:8],
                             ACT_SQRT).then_inc(sTail, 1)

            # ------------- sync: input loads, staging stores, stats DMAs,
            # BN readback
            @block.sync
            def _(sy):
                x1v = x1_d.ap().rearrange("c (h w) -> c h w", h=H)

                def x1_chunk(k):
                    r0, r1 = XBOUNDS[k], XBOUNDS[k + 1]
                    dst = AP(x1p_s, (PAD + r0) * WP + PAD,
                             [[HP * WP, C], [WP, r1 - r0], [1, W]])
                    sy.dma_start(dst, x1v[:, r0:r1, :]).then_inc(sX1, 16)

                def x0_chunk(k):
                    sl = slice(XBOUNDS[k] * W, XBOUNDS[k + 1] * W)
                    sy.dma_start(x0_s[:, sl], x0_d.ap()[:, sl]).then_inc(sX0, 16)

                # first chunks asap, then constants, then the rest interleaved
                x1_chunk(0)
                x0_chunk(0)
                sy.dma_start(gamma_s[:],
                             gamma_d.ap().rearrange("(p one) -> p one", one=1)
                             ).then_inc(sW, 16)
                sy.dma_start(beta_s[:],
                             beta_d.ap().rearrange("(p one) -> p one", one=1)
                             ).then_inc(sW, 16)
                sy.dma_start(repl_s[:], repl_d.ap()).then_inc(sW, 16)
                for k in range(1, NXC):
                    x1_chunk(k)
                    x0_chunk(k)
                for k in range(NBLK):
                    sy.wait_ge(sConvDr, 2 * SROWS * (k + 1))
                    src = stage_s[:, (k % 2) * SROWS * W:(k % 2 + 1) * SROWS * W]
                    dst = scratch_d.ap()[:, k * SROWS * W:(k + 1) * SROWS * W]
                    sy.dma_start(dst, src).then_inc(sStore if k % 2 == 0 else sStoreB,
                                                    16)
                sy.wait_ge(sV, SV_STATS)
                sy.dma_start(cc_in_d.ap().rearrange("(p two) -> p two", two=2),
                             bn_s[:, 0:2]).then_inc(sTail, 16)
                # readback into [128, 2304] BN layout, overlapped with AllReduce
                scr4 = scratch_d.ap().rearrange("o (rg g j) -> o g rg j",
                                                g=16, j=W)
                for o in range(8):
                    dst = bnin_s[16 * o:16 * (o + 1), :].rearrange(
                        "p (rg j) -> p rg j", j=W)
                    sy.dma_start(dst, scr4[o]).then_inc(sBnIn, 16)
                sy.wait_ge(sTail, 17)
                sy.dma_start(bn_s[:, 2:4],
                             cc_out_d.ap().rearrange("(p two) -> p two", two=2)
                             ).then_inc(sTail, 16)

        # ------------- BN apply tail: reuses freed arena space
        with contextlib.ExitStack() as bstack:
            eb = bstack.enter_context
            bn2_s = eb(nc.sbuf_tensor([128, 2], F32))
            bnout_s = eb(nc.sbuf_tensor([128, BNW], F32))
            block2 = eb(nc.Block())

            out4 = out_d.ap().rearrange("o (rg g j) -> o g rg j", g=16, j=W)

            def out_src(o):
                return bnout_s[16 * o:16 * (o + 1), :].rearrange(
                    "p (rg j) -> p rg j", j=W)

            @block2.scalar
            def _(s):
                s.wait_ge(sRepl, 1)
                s.activation(bn2_s[:], bn_ps[:, 0:2],
                             ACT_COPY).then_inc(sRepl, 1)
                s.wait_ge(sRepl, 2)
                s.wait_ge(sBnIn, 16 * 8)
                s.activation(bnout_s[:], bnin_s[:], ACT_RELU,
                             bias=bn2_s[:, 1:2],
                             scale=bn2_s[:, 0:1]).then_inc(sBnAct, 1)
                # half the output stores issue from the ACT HWDGE ring so the
                # two physical DGE rings drain the result in parallel
                for o in range(4, 8):
                    s.dma_start(out4[o], out_src(o)).then_inc(sBnOut, 16)

            @block2.sync
            def _(sy):
                sy.wait_ge(sBnAct, 1)
                for o in range(4):
                    sy.dma_start(out4[o], out_src(o)).then_inc(sBnOut, 16)

    nc.finalize()
    return nc


_NC_CACHE = None
LAST_EXEC_NS = None
LAST_RES = None


def kernel(x0, x1, conv_w, conv_b, gamma, beta):
    """Full inputs -> full output [8, 8, 192, 192] f32.

    conv_b is intentionally unused: training-mode BatchNorm removes any constant
    per-channel shift (mean' = mean + b exactly cancels it).
    """
    global _NC_CACHE
    x0 = np.asarray(x0, dtype=np.float32).astype(np.float16)
    x1 = np.asarray(x1, dtype=np.float32).astype(np.float16)
    conv_w = np.asarray(conv_w, dtype=np.float32)
    gamma = np.ascontiguousarray(np.asarray(gamma, dtype=np.float32))
    beta = np.ascontiguousarray(np.asarray(beta, dtype=np.float32))

    # lhsT layout [d, (ki, kj, o)]
    convw_l = np.ascontiguousarray(conv_w.transpose(1, 2, 3, 0).reshape(D, 72))
    # one-hot replication matrix: repl[k, m] = 1 iff k == m // 16
    repl = np.zeros((8, 128), dtype=np.float32)
    for kk in range(8):
        repl[kk, 16 * kk:16 * (kk + 1)] = 1.0

    if _NC_CACHE is None:
        _NC_CACHE = build_nc()
    nc = _NC_CACHE

    in_maps = []
    for c in range(NCORES):
        in_maps.append({
            "x0": np.ascontiguousarray(x0[c].reshape(C, HW)),
            "x1": np.ascontiguousarray(x1[c].reshape(C, HW)),
            "convw": convw_l,
            "gamma": gamma,
            "beta": beta,
            "repl": repl,
        })
    import os
    trace = bool(os.environ.get("KERNEL_TRACE"))
    kw = {}
    if trace:
        kw = dict(trace=True, trace_cores=[0])
    res = bass_utils.run_bass_kernel_spmd(nc, in_maps, core_ids=list(range(NCORES)),
                                          **kw)
    global LAST_EXEC_NS, LAST_RES
    LAST_RES = res
    LAST_EXEC_NS = res.exec_time_ns
    out = np.stack([res.results[c]["out"].reshape(8, H, W) for c in range(NCORES)])
    return out.astype(np.float32)


if __name__ == "__main__":
    rng = np.random.default_rng(0)
    x0 = rng.standard_normal((B, C, H, W), dtype=np.float32)
    x1 = rng.standard_normal((B, C, H, W), dtype=np.float32)
    conv_w = (rng.standard_normal((8, D, 3, 3), dtype=np.float32) * 0.05)
    conv_b = (rng.standard_normal((8,), dtype=np.float32) * 0.05)
    gamma = np.ones(8, dtype=np.float32)
    beta = np.zeros(8, dtype=np.float32)
    out = kernel(x0=x0, x1=x1, conv_w=conv_w, conv_b=conv_b, gamma=gamma, beta=beta)
    print("kernel out:", out.shape, out.dtype, float(np.abs(out).max()))


# revision 40
# speedup vs baseline: 1.3253x; 1.0071x over previous
"""Trainium2 Bass kernel: correlation(11x11,dil=2) -> Conv2d(121->8,3x3,p=1) -> BN -> ReLU.

Hardcoded problem: x0, x1 [B=8, C=128, H=192, W=192] f32. Data-parallel over batch
across 8 NeuronCores (1 sample/core); BN training-mode batch stats via a 16-float
DRAM AllReduce. See kernel() at the bottom for the host-side contract.

Per-core dataflow (row-pair pipelined over 96 pairs):
  DVE    products P[c,(row2,dj,j)] = x0[c,i,j]*x1pad[c,i+2di,j+2dj] (fp16, 1 op per
         in-range (pair,di); out-of-range row pairs are skipped entirely)
  PE     channel-sum via selector matmuls: one-hot stationary col (d%32) in col-group
         (d//32) scatters displacement d's sum to PSUM partition d; accumulating
         matmuls -> corr PSUM [121, 2*192] for rows (2p, 2p+1)
  ACT    drains corr rows into bf16 SBUF ring [121, 6*194] (zero side borders);
         skipped displacement rows stay zero (memzero at bottom edge)
  PE     conv row r: <=9 accumulating matmuls, stationary [121,8] bf16 per (ki,kj),
         rhs = ring row r+ki-1 shifted kj -> PSUM [8, 192]; interleaved between
         selector groups to avoid long PE bursts
  ACT    drains conv rows to bf16 staging (+sum via accum_out), Square pass (+sumsq)
  SYNC   staging -> DRAM scratch (bf16) every 8 rows; after all stores, readback
         into a [128, 2304] SBUF layout (p = o*16 + r%16) overlapped with the
         stats AllReduce
  tail   AllReduce [16] -> k=gamma*rsqrt(var+eps), b=beta-k*mean on 8 partitions
         (conv bias cancels in training-mode BN), replicate (k,b) to 128 partitions
         with a one-hot [8,128] matmul, then one ACT relu(k*x+b) over [128, 2304].

Inputs stream in chunks (24 rows each) so products start ~20us into the kernel;
product buffers are triple-buffered so PE conv bursts don't stall the DVE.
"""
import contextlib

import numpy as np

import concourse.bass as bass
import concourse.mybir as mybir
from concourse.ap import AP
from concourse import bass_utils

B = 8
C = 128
H = W = 192
PATCH = 11
DIL = 2
PAD = (PATCH // 2) * DIL          # 10
D = PATCH * PATCH                 # 121
HP = H + 2 * PAD                  # 212
WP = W + 2 * PAD                  # 212
HW = H * W                        # 36864
EPS = 1e-5
NCORES = 8
NPAIRS = H // 2                   # 96
RING = 6
SLOT = W + 2                      # 194
SROWS = 8                         # staging rows per DMA block
NBLK = H // SROWS                 # 24
PRODSZ = 2 * PATCH * W            # 4224 elements per product buffer (2 rows x 11 dj x 192)
PDEPTH = 3                        # product buffers (triple buffer)
# input DMA chunk row boundaries (smaller first chunks -> earlier first product)
XBOUNDS = [0, 12, 24, 48, 72, 96, 120, 144, 168, 192]
NXC = len(XBOUNDS) - 1


def _chunk_of(row):
    for ci in range(NXC):
        if row < XBOUNDS[ci + 1]:
            return ci
    raise ValueError(row)
RG = H // 16                      # 12 row-groups in the [128, RG*W] BN layout
BNW = RG * W                      # 2304

F16 = mybir.dt.float16
BF16 = mybir.dt.bfloat16
F32 = mybir.dt.float32
MULT = mybir.AluOpType.mult
SUB = mybir.AluOpType.subtract
ADD = mybir.AluOpType.add
ACT_COPY = mybir.ActivationFunctionType.Copy
ACT_SQUARE = mybir.ActivationFunctionType.Square
ACT_SQRT = mybir.ActivationFunctionType.Sqrt
ACT_IDENT = mybir.ActivationFunctionType.Identity
ACT_RELU = mybir.ActivationFunctionType.Relu
AX_X = mybir.AxisListType.X

# (pair, di) product groups whose x1 rows are in range; others are exactly zero.
ACTIVE = [(p, di) for p in range(NPAIRS) for di in range(PATCH)
          if 0 <= p + di - 5 <= NPAIRS - 1]
AIDX = {pd: k for k, pd in enumerate(ACTIVE)}
NV_ACT = len(ACTIVE)              # 1026
SV_STATS = NV_ACT + 1             # after DVE stats reduce
SV_VAR = NV_ACT + 2               # after DVE mean/var
SV_KB = NV_ACT + 3                # after DVE k/b


def _conv_taps(r):
    return [(ki, kj) for ki in range(3) for kj in range(3) if 0 <= r + ki - 1 < H]


def _skips(p):
    """(nlo, nhi): # of skipped di groups at the low/high end for pair p."""
    nlo = max(0, 5 - p)
    nhi = max(0, p - (NPAIRS - 1 - 5))
    return nlo, nhi


def build_nc():
    nc = bass.Bass("TRN2", num_devices=NCORES)

    # fp16 inputs (host-cast): halves input HBM traffic vs f32 + DMA cast
    x0_d = nc.dram_tensor("x0", [C, HW], F16, kind="ExternalInput")
    x1_d = nc.dram_tensor("x1", [C, HW], F16, kind="ExternalInput")
    convw_d = nc.dram_tensor("convw", [D, 72], F32, kind="ExternalInput")
    gamma_d = nc.dram_tensor("gamma", [8], F32, kind="ExternalInput")
    beta_d = nc.dram_tensor("beta", [8], F32, kind="ExternalInput")
    repl_d = nc.dram_tensor("repl", [8, 128], F32, kind="ExternalInput")
    out_d = nc.dram_tensor("out", [8, HW], F32, kind="ExternalOutput")
    scratch_d = nc.dram_tensor("scratch", [8, HW], BF16, kind="Internal")
    cc_in_d = nc.dram_tensor("cc_in", [16], F32, kind="Internal")
    cc_out_d = nc.dram_tensor("cc_out", [16], F32, kind="Internal",
                              addr_space="Shared")

    with contextlib.ExitStack() as outer:
        ee = outer.enter_context
        bn_s = ee(nc.sbuf_tensor([8, 16], F32))
        gamma_s = ee(nc.sbuf_tensor([8, 1], F32))
        beta_s = ee(nc.sbuf_tensor([8, 1], F32))
        repl_s = ee(nc.sbuf_tensor([8, 128], F32))
        bnin_s = ee(nc.sbuf_tensor([128, BNW], BF16))
        sX0 = ee(nc.semaphore())
        sX1 = ee(nc.semaphore())
        sW = ee(nc.semaphore())
        sV = ee(nc.semaphore())
        sMM = ee(nc.semaphore())
        sDrain = ee(nc.semaphore())
        sConvPS = ee(nc.semaphore())
        sConvDr = ee(nc.semaphore())
        sStore = ee(nc.semaphore())
        sStoreB = ee(nc.semaphore())
        sX = ee(nc.semaphore())
        sTail = ee(nc.semaphore())
        sBnIn = ee(nc.semaphore())
        sRepl = ee(nc.semaphore())
        sBnAct = ee(nc.semaphore())
        sBnOut = ee(nc.semaphore())
        with contextlib.ExitStack() as inner:
            ei = inner.enter_context
            x1p_s = ei(nc.sbuf_tensor([C, HP * WP], F16))
            x0_s = ei(nc.sbuf_tensor([C, HW], F16))
            prod_s = ei(nc.sbuf_tensor([C, PDEPTH * PRODSZ], F16))
            selz_s = ei(nc.sbuf_tensor([C, 64], F16))
            convw_s = ei(nc.sbuf_tensor([D, 72], BF16))
            ring_s = ei(nc.sbuf_tensor([D, RING * SLOT], BF16))
            stage_s = ei(nc.sbuf_tensor([8, 2 * SROWS * W], BF16))
            ssum_s = ei(nc.sbuf_tensor([8, H], F32))
            ssq_s = ei(nc.sbuf_tensor([8, H], F32))
            junk_s = ei(nc.sbuf_tensor([8, W], F32))
            cps0 = ei(nc.psum_tensor([C, 2 * W], F32))
            cps1 = ei(nc.psum_tensor([C, 2 * W], F32))
            vps0 = ei(nc.psum_tensor([8, W], F32))
            vps1 = ei(nc.psum_tensor([8, W], F32))
            bn_ps = ei(nc.psum_tensor([128, 2], F32))
            block = ei(nc.Block())
            corr_ps = [cps0, cps1]
            conv_ps = [vps0, vps1]

            # ------------- gpsimd: convw cast-DMA + tail collective
            @block.gpsimd
            def _(g):
                # convw needs an f32->bf16 cast, which only SWDGE can do
                g.dma_start(convw_s[:], convw_d.ap()).then_inc(sW, 16)
                # tail: AllReduce of per-core (sum, sumsq)
                g.wait_ge(sTail, 16)
                g.collective_compute(
                    "AllReduce", ADD,
                    replica_groups=[list(range(NCORES))],
                    ins=[cc_in_d.ap()], outs=[cc_out_d.ap()],
                ).then_inc(sTail, 1)

            # ------------- vector: memsets, products, stats/k/b math
            @block.vector
            def _(v):
                # x1p border zeros only (interior fully written by DMA chunks);
                # same-engine program order puts these before any product.
                v.memset(x1p_s[:, 0:PAD * WP], 0.0)
                v.memset(x1p_s[:, (PAD + H) * WP:HP * WP], 0.0)
                v.memset(AP(x1p_s, PAD * WP, [[HP * WP, C], [WP, H], [1, PAD]]),
                         0.0)
                v.memset(AP(x1p_s, PAD * WP + PAD + W,
                            [[HP * WP, C], [WP, H], [1, PAD]]), 0.0)
                v.memset(ring_s[:], 0.0)
                v.memset(selz_s[:, 0:32], 0.0)
                v.memset(selz_s[:, 32:33], 1.0)
                v.memset(selz_s[:, 33:64], 0.0)
                hw0 = hw1 = 0
                for k, (p, di) in enumerate(ACTIVE):
                    i = 2 * p
                    need0 = 16 * (_chunk_of(2 * p + 1) + 1)
                    need1 = 16 * (_chunk_of(2 * (p + di - 5) + 1) + 1)
                    if need0 > hw0:
                        v.wait_ge(sX0, need0)
                        hw0 = need0
                    if need1 > hw1:
                        v.wait_ge(sX1, need1)
                        hw1 = need1
                    if k >= PDEPTH:
                        v.wait_ge(sMM, k - PDEPTH + 1)
                    buf = (k % PDEPTH) * PRODSZ
                    out = AP(prod_s, buf,
                             [[PDEPTH * PRODSZ, C], [PATCH * W, 2], [W, PATCH], [1, W]])
                    in0 = AP(x0_s, i * W, [[HW, C], [W, 2], [0, PATCH], [1, W]])
                    in1 = AP(x1p_s, (i + 2 * di) * WP,
                             [[HP * WP, C], [WP, 2], [DIL, PATCH], [1, W]])
                    v.tensor_tensor(out, in0, in1, MULT).then_inc(sV, 1)
                # stats reduce
                v.wait_ge(sConvDr, 2 * H)
                v.tensor_reduce(bn_s[:, 0:1], ssum_s[:], AX_X, ADD)
                v.tensor_reduce(bn_s[:, 1:2], ssq_s[:], AX_X, ADD).then_inc(sV, 1)
                # mean/var after collective readback (self-sem chains the
                # same-engine RAW dependencies through the DVE pipeline)
                v.wait_ge(sTail, 33)
                ninv = 1.0 / float(B * HW)
                v.tensor_scalar_mul(bn_s[:, 4:5], bn_s[:, 2:3], ninv)
                v.tensor_scalar_mul(bn_s[:, 5:6], bn_s[:, 3:4], ninv).then_inc(sX, 1)
                v.wait_ge(sX, 1)
                v.tensor_tensor(bn_s[:, 6:7], bn_s[:, 4:5], bn_s[:, 4:5],
                                MULT).then_inc(sX, 1)
                v.wait_ge(sX, 2)
                v.tensor_tensor(bn_s[:, 7:8], bn_s[:, 5:6], bn_s[:, 6:7],
                                SUB).then_inc(sX, 1)
                v.wait_ge(sX, 3)
                v.tensor_scalar_add(bn_s[:, 7:8], bn_s[:, 7:8],
                                    EPS).then_inc(sV, 1)
                # k, b after ACT sqrt
                v.wait_ge(sTail, 34)
                v.wait_ge(sW, 64)   # gamma/beta loaded (long since)
                v.reciprocal(bn_s[:, 9:10], bn_s[:, 8:9]).then_inc(sX, 1)
                v.wait_ge(sX, 4)
                v.tensor_tensor(bn_s[:, 10:11], gamma_s[:], bn_s[:, 9:10],
                                MULT).then_inc(sX, 1)
                v.wait_ge(sX, 5)
                v.tensor_tensor(bn_s[:, 13:14], bn_s[:, 10:11], bn_s[:, 4:5],
                                MULT).then_inc(sX, 1)
                v.wait_ge(sX, 6)
                v.tensor_tensor(bn_s[:, 11:12], beta_s[:], bn_s[:, 13:14],
                                SUB).then_inc(sV, 1)

            # ------------- tensor: selector matmuls + conv matmuls + k/b repl
            @block.tensor
            def _(t):
                conv_started = [False]

                def emit_conv(r):
                    if not conv_started[0]:
                        t.wait_ge(sW, 64)   # convw (and other W-group DMAs) done
                        conv_started[0] = True
                    t.wait_ge(sDrain, min(r + 2, H))
                    if r >= 2:
                        t.wait_ge(sConvDr, 2 * (r - 1))
                    pp = conv_ps[r % 2]
                    taps = _conv_taps(r)
                    mm = None
                    for n, (ki, kj) in enumerate(taps):
                        src = r + ki - 1
                        wcol = (ki * 3 + kj) * 8
                        lhsT = convw_s[:, wcol:wcol + 8]
                        rhs = AP(ring_s, (src % RING) * SLOT + kj,
                                 [[RING * SLOT, D], [1, W]])
                        mm = t.matmul(pp[:, :], lhsT, rhs,
                                      start=(n == 0), stop=(n == len(taps) - 1))
                    mm.then_inc(sConvPS, 1)

                for p in range(NPAIRS):
                    if p >= 2:
                        t.wait_ge(sDrain, 2 * p - 2)   # corr psum pp drained
                    started = set()
                    nlo, nhi = _skips(p)
                    dmin, dmax = PATCH * nlo, PATCH * (PATCH - nhi) - 1
                    # last computed displacement per PSUM col-group closes its
                    # accumulation group
                    last_per_cg = {}
                    for d in range(dmin, dmax + 1):
                        last_per_cg[d // 32] = d
                    for di in range(nlo, PATCH - nhi):
                        k = AIDX[(p, di)]
                        t.wait_ge(sV, k + 1)
                        if di == nlo:
                            # col-groups with no computed displacement still get
                            # drained (full-range PSUM read): write zeros via a
                            # matmul with the all-zero selector as stationary.
                            # After the sV wait, x0_s chunk 0 is loaded, so the
                            # (ignored, x0) rhs values are finite.
                            for cg in range(4):
                                if not (dmin <= 32 * cg + 31 and 32 * cg <= dmax):
                                    t.matmul(corr_ps[p % 2][32 * cg:32 * (cg + 1), :],
                                             selz_s[:, 0:32], x0_s[:, 0:2 * W],
                                             start=True, stop=True,
                                             tile_position=(0, 32 * cg))
                        buf = (k % PDEPTH) * PRODSZ
                        mm = None
                        for dj in range(PATCH):
                            d = di * PATCH + dj
                            cg, m = d // 32, d % 32
                            sel = selz_s[:, 32 - m:64 - m]
                            rhs = AP(prod_s, buf + dj * W,
                                     [[PDEPTH * PRODSZ, C], [PATCH * W, 2], [1, W]])
                            out = corr_ps[p % 2][32 * cg:32 * (cg + 1), :]
                            mm = t.matmul(out, sel, rhs,
                                          start=(cg not in started),
                                          stop=(d == last_per_cg[cg]),
                                          tile_position=(0, 32 * cg))
                            started.add(cg)
                        mm.then_inc(sMM, 1)
                        # interleave conv rows mid-pair to avoid PE bursts
                        if di == nlo + 4:
                            r = 2 * p - 4
                            if 0 <= r:
                                emit_conv(r)
                    r = 2 * p - 3
                    if 0 <= r:
                        emit_conv(r)
                for r in range(2 * NPAIRS - 4, H):
                    emit_conv(r)
                # k,b [8,2] -> [128,2] replication via one-hot [8,128] matmul
                t.wait_ge(sV, SV_KB)
                t.matmul(bn_ps[:, 0:2], repl_s[:], bn_s[:, 10:12],
                         start=True, stop=True).then_inc(sRepl, 1)

            # ------------- scalar: corr drains, conv drains + stats, sqrt
            @block.scalar
            def _(s):
                def conv_drain(r):
                    s.wait_ge(sConvPS, r + 1)
                    if r >= 1:
                        s.wait_ge(sConvDr, 2 * r)   # order junk/stage WAW
                    if r % SROWS == 0 and r >= 2 * SROWS:
                        k = r // SROWS
                        s.wait_ge(sStore if k % 2 == 0 else sStoreB,
                                  16 * (k // 2))
                    base = ((r // SROWS) % 2) * SROWS * W + (r % SROWS) * W
                    s.activation(stage_s[:, base:base + W], conv_ps[r % 2][:, :],
                                 ACT_COPY,
                                 accum_out=ssum_s[:, r:r + 1]).then_inc(sConvDr, 1)
                    s.activation(junk_s[:], conv_ps[r % 2][:, :], ACT_SQUARE,
                                 accum_out=ssq_s[:, r:r + 1]).then_inc(sConvDr, 1)

                mm_done = 0
                for p in range(NPAIRS):
                    i = 2 * p
                    nlo, nhi = _skips(p)
                    mm_done += PATCH - nlo - nhi
                    s.wait_ge(sMM, mm_done)
                    for k in range(2):
                        r = i + k
                        if r >= RING:
                            s.wait_ge(sConvPS, r - 4)   # ring slot reuse
                        dst = AP(ring_s, (r % RING) * SLOT + 1,
                                 [[RING * SLOT, D], [1, W]])
                        s.activation(dst, corr_ps[p % 2][0:D, k * W:(k + 1) * W],
                                     ACT_COPY).then_inc(sDrain, 1)
                    for r in (2 * p - 4, 2 * p - 3):
                        if 0 <= r:
                            conv_drain(r)
                for r in range(2 * NPAIRS - 4, H):
                    conv_drain(r)
                # sqrt(var + eps)
                s.wait_ge(sV, SV_VAR)
                s.activation(bn_s[:, 8:9], bn_s[:, 7:8],
                             ACT_SQRT).then_inc(sTail, 1)

            # ------------- sync: input loads, staging stores, stats DMAs,
            # BN readback
            @block.sync
            def _(sy):
                x1v = x1_d.ap().rearrange("c (h w) -> c h w", h=H)

                def x1_chunk(k):
                    r0, r1 = XBOUNDS[k], XBOUNDS[k + 1]
                    dst = AP(x1p_s, (PAD + r0) * WP + PAD,
                             [[HP * WP, C], [WP, r1 - r0], [1, W]])
                    sy.dma_start(dst, x1v[:, r0:r1, :]).then_inc(sX1, 16)

                def x0_chunk(k):
                    sl = slice(XBOUNDS[k] * W, XBOUNDS[k + 1] * W)
                    sy.dma_start(x0_s[:, sl], x0_d.ap()[:, sl]).then_inc(sX0, 16)

                # first chunks asap, then constants, then the rest interleaved
                x1_chunk(0)
                x0_chunk(0)
                sy.dma_start(gamma_s[:],
                             gamma_d.ap().rearrange("(p one) -> p one", one=1)
                             ).then_inc(sW, 16)
                sy.dma_start(beta_s[:],
                             beta_d.ap().rearrange("(p one) -> p one", one=1)
                             ).then_inc(sW, 16)
                sy.dma_start(repl_s[:], repl_d.ap()).then_inc(sW, 16)
                for k in range(1, NXC):
                    x1_chunk(k)
                    x0_chunk(k)
                for k in range(NBLK):
                    sy.wait_ge(sConvDr, 2 * SROWS * (k + 1))
                    src = stage_s[:, (k % 2) * SROWS * W:(k % 2 + 1) * SROWS * W]
                    dst = scratch_d.ap()[:, k * SROWS * W:(k + 1) * SROWS * W]
                    sy.dma_start(dst, src).then_inc(sStore if k % 2 == 0 else sStoreB,
                                                    16)
                sy.wait_ge(sV, SV_STATS)
                sy.dma_start(cc_in_d.ap().rearrange("(p two) -> p two", two=2),
                             bn_s[:, 0:2]).then_inc(sTail, 16)
                # readback into [128, 2304] BN layout, overlapped with AllReduce
                scr4 = scratch_d.ap().rearrange("o (rg g j) -> o g rg j",
                                                g=16, j=W)
                for o in range(8):
                    dst = bnin_s[16 * o:16 * (o + 1), :].rearrange(
                        "p (rg j) -> p rg j", j=W)
                    sy.dma_start(dst, scr4[o]).then_inc(sBnIn, 16)
                sy.wait_ge(sTail, 17)
                sy.dma_start(bn_s[:, 2:4],
                             cc_out_d.ap().rearrange("(p two) -> p two", two=2)
                             ).then_inc(sTail, 16)

        # ------------- BN apply tail: reuses freed arena space
        with contextlib.ExitStack() as bstack:
            eb = bstack.enter_context
            bn2_s = eb(nc.sbuf_tensor([128, 2], F32))
            bnout_s = eb(nc.sbuf_tensor([128, BNW], F32))
            block2 = eb(nc.Block())

            out4 = out_d.ap().rearrange("o (rg g j) -> o g rg j", g=16, j=W)

            def out_src(o):
                return bnout_s[16 * o:16 * (o + 1), :].rearrange(
                    "p (rg j) -> p rg j", j=W)

            @block2.scalar
            def _(s):
                s.wait_ge(sRepl, 1)
                s.activation(bn2_s[:], bn_ps[:, 0:2],
                             ACT_COPY).then_inc(sRepl, 1)
                s.wait_ge(sRepl, 2)
                s.wait_ge(sBnIn, 16 * 8)
                s.activation(bnout_s[:], bnin_s[:], ACT_RELU,
                             bias=bn2_s[:, 1:2],
                             scale=bn2_s[:, 0:1]).then_inc(sBnAct, 1)
                # half the output stores issue from the ACT HWDGE ring so the
                # two physical DGE rings drain the result in parallel
                for o in range(4, 8):
                    s.dma_start(out4[o], out_src(o)).then_inc(sBnOut, 16)

            @block2.sync
            def _(sy):
                sy.wait_ge(sBnAct, 1)
                for o in range(4):
                    sy.dma_start(out4[o], out_src(o)).then_inc(sBnOut, 16)

    nc.finalize()
    return nc


_NC_CACHE = None
LAST_EXEC_NS = None
LAST_RES = None


def kernel(x0, x1, conv_w, conv_b, gamma, beta):
    """Full inputs -> full output [8, 8, 192, 192] f32.

    conv_b is intentionally unused: training-mode BatchNorm removes any constant
    per-channel shift (mean' = mean + b exactly cancels it).
    """
    global _NC_CACHE
    x0 = np.asarray(x0, dtype=np.float32).astype(np.float16)
    x1 = np.asarray(x1, dtype=np.float32).astype(np.float16)
    conv_w = np.asarray(conv_w, dtype=np.float32)
    gamma = np.ascontiguousarray(np.asarray(gamma, dtype=np.float32))
    beta = np.ascontiguousarray(np.asarray(beta, dtype=np.float32))

    # lhsT layout [d, (ki, kj, o)]
    convw_l = np.ascontiguousarray(conv_w.transpose(1, 2, 3, 0).reshape(D, 72))
    # one-hot replication matrix: repl[k, m] = 1 iff k == m // 16
    repl = np.zeros((8, 128), dtype=np.float32)
    for kk in range(8):
        repl[kk, 16 * kk:16 * (kk + 1)] = 1.0

    if _NC_CACHE is None:
        _NC_CACHE = build_nc()
    nc = _NC_CACHE

    in_maps = []
    for c in range(NCORES):
        in_maps.append({
            "x0": np.ascontiguousarray(x0[c].reshape(C, HW)),
            "x1": np.ascontiguousarray(x1[c].reshape(C, HW)),
            "convw": convw_l,
            "gamma": gamma,
            "beta": beta,
            "repl": repl,
        })
    import os
    trace = bool(os.environ.get("KERNEL_TRACE"))
    kw = {}
    if trace:
        kw = dict(trace=True, trace_cores=[0])
    res = bass_utils.run_bass_kernel_spmd(nc, in_maps, core_ids=list(range(NCORES)),
                                          **kw)
    global LAST_EXEC_NS, LAST_RES
    LAST_RES = res
    LAST_EXEC_NS = res.exec_time_ns
    out = np.stack([res.results[c]["out"].reshape(8, H, W) for c in range(NCORES)])
    return out.astype(np.float32)


if __name__ == "__main__":
    rng = np.random.default_rng(0)
    x0 = rng.standard_normal((B, C, H, W), dtype=np.float32)
    x1 = rng.standard_normal((B, C, H, W), dtype=np.float32)
    conv_w = (rng.standard_normal((8, D, 3, 3), dtype=np.float32) * 0.05)
    conv_b = (rng.standard_normal((8,), dtype=np.float32) * 0.05)
    gamma = np.ones(8, dtype=np.float32)
    beta = np.zeros(8, dtype=np.float32)
    out = kernel(x0=x0, x1=x1, conv_w=conv_w, conv_b=conv_b, gamma=gamma, beta=beta)
    print("kernel out:", out.shape, out.dtype, float(np.abs(out).max()))
